# revision 59
# baseline (speedup 1.0000x reference)
"""Trainium2 Bass kernel for nn_ClusterGCN (3-layer 2-edge-type GCN + heads).

Strategy (8 NeuronCores, node-parallel):
  - Nodes sharded contiguously: core c owns rows [c*S, (c+1)*S), S = N/8.
  - ONE replicated node-major fp16 table of h per layer (not per edge type);
    the full GCN edge coefficient dinv[src]*dinv[dst] is folded into the
    one-hot scatter matrix, so a single AllGather per layer boundary
    suffices. Edges are sharded by dst; messages h[src] are fetched with
    GPSIMD dma_gather (int16 indices, lo/hi base split for N > 32768) and
    scatter-added into a feature-major accumulator via one-hot matmuls on
    the PE: out[feat, dst_slot] += msg[edge, feat]^T @ (coef * onehot).
  - g_t = scatter_t + dinv_t^2 * h adds the self-loop, then
    h' = BN(g0 @ W0 + g1 @ W1) with batch stats AllReduced across cores.
  - Next-layer tables are built with a single XBAR dma transpose of the
    normalized h16 and AllGathered. Heads (tanh/relu/l2norm MLPs) run
    node-sharded in two activation-table passes (tanh pass, sqrt pass).
"""
import math
import numpy as np
import ml_dtypes

import concourse.bacc as bacc
import concourse.bass as bass
import concourse.mybir as mybir
import concourse.tile as tile
from concourse.library_config import mlp as mlp_lib
from concourse.bass_utils import run_bass_kernel_spmd

NCORES = 8
D = 128
L = 3
EPS_BN = 1e-5
EPS_NORM = 1e-12
IDX_LIMIT = 32768
PIECE = 1024          # gather slots per dma_gather instruction (ring sized to fit)
SENT_DST = 320.0      # sentinel dst slot (fp16-exact, >= 128)

f32 = mybir.dt.float32
bf16 = mybir.dt.float16  # (fp16 everywhere: 8x finer mantissa than bf16, same HW rates)
i16 = mybir.dt.int16
AF = mybir.ActivationFunctionType
OP = mybir.AluOpType


# ---------------------------------------------------------------- host prep

def _prep_type(edge_index, N, S):
    """Per edge type: degrees + per-core common-shape gather/scatter schedule."""
    src = np.asarray(edge_index[0], np.int64)
    dst = np.asarray(edge_index[1], np.int64)
    deg = np.bincount(dst, minlength=N).astype(np.float64) + 1.0
    dinv = 1.0 / np.sqrt(deg)

    NW = (S + 127) // 128
    HI_BASE = N - IDX_LIMIT  # hi-region table base; rows [HI_BASE, N)
    # src in [0, IDX_LIMIT) reachable from region 0; [HI_BASE, N) from region 1.
    # srcs in the overlap [max(HI_BASE,0), IDX_LIMIT) are flexible - used to
    # round region-0 groups up to full chunks and minimize sentinel padding.
    cores = []
    for c in range(NCORES):
        m = (dst >= c * S) & (dst < (c + 1) * S)
        s_c = src[m]
        d_c = dst[m]
        dl = d_c - c * S
        w = dl // 128
        order = np.lexsort((s_c, dl, w))
        cores.append((s_c[order], dl[order], w[order], d_c[order]))

    if N > IDX_LIMIT:
        K = np.zeros((2, NW), np.int64)
        must_lo = []
        for (s_c, dl, w, _) in cores:
            cnt_lo = np.bincount(w[s_c < HI_BASE], minlength=NW)
            must_lo.append(cnt_lo)
            K[0] = np.maximum(K[0], (cnt_lo + 127) // 128)
        K[0] = np.maximum(K[0], 1)
        core_reg = []
        for ci, (s_c, dl, w, _) in enumerate(cores):
            reg = (s_c >= IDX_LIMIT).astype(np.int64)
            for w_ in range(NW):
                cap = K[0][w_] * 128
                flex = np.flatnonzero((w == w_) & (s_c >= HI_BASE) & (s_c < IDX_LIMIT))
                take = min(max(cap - int(must_lo[ci][w_]), 0), len(flex))
                reg[flex[:take]] = 0
                reg[flex[take:]] = 1
            cnt_hi = np.bincount(w[reg == 1], minlength=NW)
            K[1] = np.maximum(K[1], (cnt_hi + 127) // 128)
            core_reg.append(reg)
        K[1] = np.maximum(K[1], 1)
        cores = [(s_c, dl, w, d_c, core_reg[ci])
                 for ci, (s_c, dl, w, d_c) in enumerate(cores)]
    else:
        K = np.zeros((2, NW), np.int64)
        for (s_c, dl, w, _) in cores:
            cnt = np.bincount(w, minlength=NW)
            K[0] = np.maximum(K[0], (cnt + 127) // 128)
        K[0] = np.maximum(K[0], 1)
        cores = [(s_c, dl, w, d_c, np.zeros(len(s_c), np.int64))
                 for (s_c, dl, w, d_c) in cores]

    schedule = []  # (region, window, nchunks) in slot order
    for r in (0, 1):
        for w_ in range(NW):
            if K[r][w_] > 0:
                schedule.append((r, int(w_), int(K[r][w_])))
    nchunks = sum(k for _, _, k in schedule)
    stot = nchunks * 128

    idx_all = np.zeros((NCORES, max(stot, 128)), np.int64)
    rel_all = np.full((NCORES, max(nchunks, 1) * 128), SENT_DST, np.float64)
    coef_all = np.zeros((NCORES, max(nchunks, 1) * 128), np.float64)
    for ci, (s_c, dl, w, d_c, reg) in enumerate(cores):
        pos = 0
        for (r, w_, k) in schedule:
            m = (reg == r) & (w == w_)
            n = int(m.sum())
            sv = s_c[m]
            idx_all[ci, pos:pos + n] = sv if r == 0 else sv - (N - IDX_LIMIT)
            rel_all[ci, pos:pos + n] = dl[m] - w_ * 128
            coef_all[ci, pos:pos + n] = dinv[sv] * dinv[d_c[m]]
            pos += k * 128

    # wrapped int16 index layout: idxs[p, s] = idx[s*16 + p%16]
    cols = max(stot // 16, 1)
    idx_w = np.zeros((NCORES, 128, cols), np.int16)
    if stot:
        a = idx_all[:, :stot].reshape(NCORES, cols, 16)  # [c, s, j]
        for p in range(128):
            idx_w[:, p, :] = a[:, :, p % 16]
    # dst-slot / coefficient tiles: [p, chunk] = value of edge chunk*128+p
    rel_t = np.ascontiguousarray(
        rel_all[:, :nchunks * 128].reshape(NCORES, nchunks, 128).transpose(0, 2, 1)
    ).astype(np.float32)
    coef_t = np.ascontiguousarray(
        coef_all[:, :nchunks * 128].reshape(NCORES, nchunks, 128).transpose(0, 2, 1)
    ).astype(np.float32)

    dinvsq = (dinv * dinv).astype(np.float32)
    return dinv.astype(np.float32), dinvsq, schedule, idx_w, rel_t, coef_t, stot, nchunks


def _pieces(schedule):
    """Split slot range into gather pieces that do not cross the lo/hi boundary.
    Returns list of (slot_start, slot_count, region)."""
    out = []
    for r in (0, 1):
        lo = sum(k * 128 for (rr, _, k) in schedule if rr < r)
        n = sum(k * 128 for (rr, _, k) in schedule if rr == r)
        p = lo
        while p < lo + n:
            c = min(PIECE, lo + n - p)
            out.append((p, c, r))
            p += c
    return out


# ---------------------------------------------------------------- device build

def _build(N, S, sch0, stot0, nch0, sch1, stot1, nch1):
    NW = (S + 127) // 128
    SP_ = NW * 128           # padded S (multiple of 128) for the XBAR transpose
    NF = (S + 511) // 512    # 512-wide node tiles
    nc = bacc.Bacc("TRN2", target_bir_lowering=False, debug=False,
                   num_devices=NCORES, dynamic_dma_scratch_size=PIECE * 16)

    def din(name, shape, dt):
        return nc.dram_tensor(name, shape, dt, kind="ExternalInput")

    tab_in = din("tab_in", [N, D], bf16)
    xT_in = din("xT_in", [128, S], bf16)
    dinvsq_in = [din("dinvsq0_in", [128, S], bf16), din("dinvsq1_in", [128, S], bf16)]
    idx_in = [din("idx0_in", [128, max(stot0 // 16, 1)], i16),
              din("idx1_in", [128, max(stot1 // 16, 1)], i16)]
    rel_in = [din("rel0_in", [128, max(nch0, 1)], f32),
              din("rel1_in", [128, max(nch1, 1)], f32)]
    coef_in = [din("coef0_in", [128, max(nch0, 1)], f32),
               din("coef1_in", [128, max(nch1, 1)], f32)]
    wd_in = din("wd_in", [L * 2 * 128, D], bf16)
    gb_in = din("gb_in", [128, 2 * L], f32)
    wh_in = din("wh_in", [6 * 128, D], bf16)
    hb_in = din("hb_in", [128, 6], f32)
    iota_in = din("iota_in", [128, 128], bf16)
    ones_in = din("ones_in", [128, 128], bf16)

    outs = [nc.dram_tensor(n, [128, S], bf16, kind="ExternalOutput")
            for n in ("e1_o", "e2_o", "p1_o", "p2_o")]

    with tile.TileContext(nc) as tc:
        with (
            tc.tile_pool(name="const", bufs=1) as const,
            tc.tile_pool(name="g", bufs=1) as gpool,
            tc.tile_pool(name="msg", bufs=8) as msgp,
            tc.tile_pool(name="oh", bufs=64) as ohp,
            tc.tile_pool(name="scr", bufs=1) as scp,
            tc.tile_pool(name="psA", bufs=4, space="PSUM") as psA,
            tc.tile_pool(name="psB", bufs=4, space="PSUM") as psB,
            tc.tile_pool(name="dram", bufs=1, space="DRAM") as dram,
        ):
            nc.gpsimd.load_library(mlp_lib)

            # ---- persistent SBUF tiles
            iota_t = const.tile([128, 128], bf16)
            ones_t = const.tile([128, 128], bf16)
            dinvsq_t = [const.tile([128, S], bf16, tag=f"dq{t}", name=f"dq{t}")
                        for t in (0, 1)]
            idx_t = [const.tile([128, max(stot0 // 16, 1)], i16, tag="idx0", name="idx0"),
                     const.tile([128, max(stot1 // 16, 1)], i16, tag="idx1", name="idx1")]
            rel_t = [const.tile([128, max(nch0, 1)], f32, tag="rel0", name="rel0"),
                     const.tile([128, max(nch1, 1)], f32, tag="rel1", name="rel1")]
            coef_t = [const.tile([128, max(nch0, 1)], f32, tag="coef0", name="coef0"),
                      const.tile([128, max(nch1, 1)], f32, tag="coef1", name="coef1")]
            wd_t = const.tile([128, L * 2, D], bf16)     # dense weights
            wh_t = const.tile([128, 6, D], bf16)         # head weights
            gb_t = const.tile([128, 2 * L], f32)
            hb_t = const.tile([128, 6], f32)
            epsn_t = const.tile([128, 1], f32)
            nc.vector.memset(epsn_t[:], EPS_NORM * EPS_NORM)

            h16_t = gpool.tile([128, SP_], bf16, tag="h16")      # padded cols
            h_pre = gpool.tile([128, S], f32, tag="hpre")
            gbf_t = [gpool.tile([128, S], bf16, tag=f"gbf{t}", name=f"gbf{t}")
                     for t in (0, 1)]
            # stage (layer boundaries) and e1b (heads) never live at the same
            # time; alias them in one padded buffer to save SBUF.
            ub = gpool.tile([128, SP_], bf16, tag="stage")
            stage = ub[:].rearrange("p (w k) -> p w k", k=128)
            e1b_t = ub[:]
            t2b_t = gpool.tile([128, S], bf16, tag="t2b")

            nc.sync.dma_start(iota_t[:], iota_in[:])
            nc.sync.dma_start(ones_t[:], ones_in[:])
            for t in (0, 1):
                nc.sync.dma_start(dinvsq_t[t][:], dinvsq_in[t][:])
                nc.sync.dma_start(idx_t[t][:], idx_in[t][:])
                nc.sync.dma_start(rel_t[t][:], rel_in[t][:])
                nc.sync.dma_start(coef_t[t][:], coef_in[t][:])
            nc.sync.dma_start(
                wd_t[:], wd_in[:].rearrange("(k p) d -> p k d", p=128))
            nc.sync.dma_start(
                wh_t[:], wh_in[:].rearrange("(k p) d -> p k d", p=128))
            nc.sync.dma_start(gb_t[:], gb_in[:])
            nc.sync.dma_start(hb_t[:], hb_in[:])
            nc.sync.dma_start(h16_t[:, :S], xT_in[:])
            if SP_ > S:
                nc.vector.memset(h16_t[:, S:], 0.0)

            # ---- internal DRAM for collectives
            ag_in = {}
            ag_out = {}
            for l in (0, 1):
                ag_in[l] = dram.tile([S, D], bf16, tag=f"agi{l}", name=f"agi{l}")
                ag_out[l] = dram.tile([N, D], bf16, addr_space="Shared",
                                      tag=f"ago{l}", name=f"ago{l}")
            st_in = [dram.tile([128, 2], f32, tag=f"sti{l}", name=f"sti{l}")
                     for l in range(L)]
            st_out = [dram.tile([128, 2], f32, addr_space="Shared",
                                tag=f"sto{l}", name=f"sto{l}") for l in range(L)]

            schs = (sch0, sch1)
            rg = [list(range(NCORES))]

            for l in range(L):
                # ---------------- scatter phase (both edge types)
                for t in (0, 1):
                    if l == 0:
                        tab_lo = tab_in[:]
                        tab_hi = tab_in[N - IDX_LIMIT:] if N > IDX_LIMIT else None
                    else:
                        tab_lo = ag_out[l - 1][:]
                        tab_hi = ag_out[l - 1][N - IDX_LIMIT:] \
                            if N > IDX_LIMIT else None

                    sch = schs[t]
                    # chunk meta: (region, window, win_first, win_last,
                    #              group_first, group_last); groups = 4 windows
                    # of one region sharing a [128,512] PSUM bank.
                    chunk_meta = []
                    for si, (r, w_, k) in enumerate(sch):
                        gf = (w_ % 4 == 0) or si == 0 or sch[si - 1][0] != r
                        gl = (w_ % 4 == 3) or si == len(sch) - 1 \
                            or sch[si + 1][0] != r
                        for j in range(k):
                            chunk_meta.append(
                                (r, w_, j == 0, j == k - 1,
                                 gf and j == 0, gl and j == k - 1))

                    pieces = _pieces(sch)
                    acc = None
                    for (p0, cnt, r) in pieces:
                        msg = msgp.tile([128, PIECE // 128, 128], bf16,
                                        tag="msg")
                        src_ap = tab_lo if r == 0 else tab_hi
                        nc.gpsimd.dma_gather(
                            msg[:, :cnt // 128, :], src_ap,
                            idx_t[t][:, p0 // 16:(p0 + cnt) // 16],
                            num_idxs=cnt, num_idxs_reg=cnt, elem_size=D,
                        )
                        for ci in range(cnt // 128):
                            gc = p0 // 128 + ci
                            (cr, w_, first, last, gfirst, glast) = chunk_meta[gc]
                            oh = ohp.tile([128, 128], bf16, tag="oh")
                            nc.vector.tensor_scalar(
                                out=oh[:], in0=iota_t[:],
                                scalar1=rel_t[t][:, gc:gc + 1],
                                scalar2=coef_t[t][:, gc:gc + 1],
                                op0=OP.is_equal, op1=OP.mult,
                            )
                            if gfirst:
                                acc = psA.tile([128, 512], f32, space="PSUM",
                                               tag="sc")
                            ws = (w_ % 4) * 128
                            nc.tensor.matmul(out=acc[:, ws:ws + 128],
                                             lhsT=msg[:, ci, :],
                                             rhs=oh[:], start=first, stop=last)
                            if glast:
                                base = (w_ // 4) * 512
                                wd = min(512, S - base)
                                # evacuate scatter PSUM; region 0 initializes
                                # gbf (Act, overwrite), region 1 accumulates
                                # on top (DVE add). Self-loop joins at dense.
                                if cr == 0:
                                    nc.scalar.activation(
                                        out=gbf_t[t][:, base:base + wd],
                                        in_=acc[:, :wd], func=AF.Identity)
                                else:
                                    nc.vector.tensor_tensor(
                                        out=gbf_t[t][:, base:base + wd],
                                        in0=gbf_t[t][:, base:base + wd],
                                        in1=acc[:, :wd], op=OP.add)

                # ---------------- dense + stats partials
                sum_p = scp.tile([128, NF], f32, tag="sump")
                ssq_p = scp.tile([128, NF], f32, tag="ssqp")
                for ft in range(NF):
                    fw = min(512, S - ft * 512)
                    sl = slice(ft * 512, ft * 512 + fw)
                    # self-loop tiles: sl_t = dinv^2 * h  (fp16, 4x DVE)
                    slt = [scp.tile([128, 512], bf16, tag=f"sl{t}", bufs=3,
                                    name=f"sl{t}") for t in (0, 1)]
                    for t in (0, 1):
                        nc.vector.tensor_tensor(out=slt[t][:, :fw],
                                                in0=h16_t[:, sl],
                                                in1=dinvsq_t[t][:, sl],
                                                op=OP.mult)
                    dp = psB.tile([128, 512], f32, space="PSUM", tag="dense")
                    nc.tensor.matmul(out=dp[:, :fw], lhsT=wd_t[:, l * 2, :],
                                     rhs=gbf_t[0][:, sl], start=True, stop=False)
                    nc.tensor.matmul(out=dp[:, :fw], lhsT=wd_t[:, l * 2 + 1, :],
                                     rhs=gbf_t[1][:, sl], start=False, stop=False)
                    nc.tensor.matmul(out=dp[:, :fw], lhsT=wd_t[:, l * 2, :],
                                     rhs=slt[0][:, :fw], start=False, stop=False)
                    nc.tensor.matmul(out=dp[:, :fw], lhsT=wd_t[:, l * 2 + 1, :],
                                     rhs=slt[1][:, :fw], start=False, stop=True)
                    nc.scalar.activation(out=h_pre[:, sl], in_=dp[:, :fw],
                                         func=AF.Identity,
                                         accum_out=sum_p[:, ft:ft + 1])
                    sq = scp.tile([128, 512], bf16, tag="sq", bufs=3)
                    nc.scalar.activation(out=sq[:, :fw], in_=dp[:, :fw],
                                         func=AF.Square,
                                         accum_out=ssq_p[:, ft:ft + 1])

                # ---------------- BN stats allreduce
                st = scp.tile([128, 2], f32, tag="st")
                nc.vector.tensor_reduce(out=st[:, 0:1], in_=sum_p[:],
                                        axis=mybir.AxisListType.X, op=OP.add)
                nc.vector.tensor_reduce(out=st[:, 1:2], in_=ssq_p[:],
                                        axis=mybir.AxisListType.X, op=OP.add)
                nc.sync.dma_start(st_in[l][:], st[:])
                nc.gpsimd.collective_compute(
                    "AllReduce", OP.add, replica_groups=rg,
                    ins=[st_in[l].opt()], outs=[st_out[l].opt()])
                sta = scp.tile([128, 2], f32, tag="sta")
                nc.sync.dma_start(sta[:], st_out[l][:])

                mean = scp.tile([128, 1], f32, tag="mean")
                var = scp.tile([128, 1], f32, tag="var")
                scl = scp.tile([128, 1], f32, tag="scl")
                sht = scp.tile([128, 1], f32, tag="sht")
                tmp = scp.tile([128, 1], f32, tag="tmp1")
                inv_n = 1.0 / float(N)
                nc.vector.tensor_scalar(out=mean[:], in0=sta[:, 0:1],
                                        scalar1=inv_n, scalar2=None, op0=OP.mult)
                nc.vector.tensor_scalar(out=var[:], in0=sta[:, 1:2],
                                        scalar1=inv_n, scalar2=None, op0=OP.mult)
                nc.vector.tensor_tensor(out=tmp[:], in0=mean[:], in1=mean[:],
                                        op=OP.mult)
                nc.vector.tensor_tensor(out=var[:], in0=var[:], in1=tmp[:],
                                        op=OP.subtract)
                # scl = gamma / sqrt(var + eps); sht = beta - mean*scl
                nc.vector.tensor_scalar(out=var[:], in0=var[:], scalar1=EPS_BN,
                                        scalar2=None, op0=OP.add)
                nc.scalar.activation(out=tmp[:], in_=var[:], func=AF.Sqrt)
                nc.vector.reciprocal(out=tmp[:], in_=tmp[:])
                nc.vector.tensor_tensor(out=scl[:], in0=gb_t[:, l:l + 1],
                                        in1=tmp[:], op=OP.mult)
                nc.vector.tensor_tensor(out=tmp[:], in0=mean[:], in1=scl[:],
                                        op=OP.mult)
                nc.vector.tensor_tensor(out=sht[:], in0=gb_t[:, L + l:L + l + 1],
                                        in1=tmp[:], op=OP.subtract)

                # ---------------- normalize (+ relu except last layer)
                if l < L - 1:
                    nc.scalar.activation(out=h16_t[:, :S], in_=h_pre[:],
                                         func=AF.Relu,
                                         bias=sht[:], scale=scl[:])
                else:
                    # tiled so the heads pipeline behind the normalize
                    for ft in range(NF):
                        fw = min(512, S - ft * 512)
                        sl = slice(ft * 512, ft * 512 + fw)
                        nc.scalar.activation(out=h16_t[:, sl],
                                             in_=h_pre[:, sl],
                                             func=AF.Identity,
                                             bias=sht[:], scale=scl[:])

                # ---------------- next-layer table: XBAR transpose + allgather
                if l < L - 1:
                    nc.sync.dma_start_transpose(stage[:], h16_t[:])
                    full = (S // 128) * 128
                    nc.sync.dma_start(
                        ag_in[l][:full].rearrange("(w p) d -> p w d", p=128),
                        stage[:, :S // 128, :])
                    if S > full:
                        nc.sync.dma_start(ag_in[l][full:],
                                          stage[:S - full, S // 128, :])
                    nc.gpsimd.collective_compute(
                        "AllGather", OP.bypass, replica_groups=rg,
                        ins=[ag_in[l].opt()], outs=[ag_out[l].opt()])

            # ---------------- heads (two activation-table passes)
            # pass 1: tanh embeddings, written straight to fp16 buffers
            for ft in range(NF):
                fw = min(512, S - ft * 512)
                sl = slice(ft * 512, ft * 512 + fw)
                e1p = psB.tile([128, 512], f32, space="PSUM", tag="dense")
                nc.tensor.matmul(out=e1p[:, :fw], lhsT=wh_t[:, 0, :],
                                 rhs=h16_t[:, sl], start=True, stop=True)
                nc.scalar.activation(out=e1b_t[:, sl], in_=e1p[:, :fw],
                                     func=AF.Tanh, bias=hb_t[:, 0:1])
                nc.sync.dma_start(outs[0][:, sl], e1b_t[:, sl])
                e2p = psB.tile([128, 512], f32, space="PSUM", tag="dense")
                nc.tensor.matmul(out=e2p[:, :fw], lhsT=wh_t[:, 1, :],
                                 rhs=h16_t[:, sl], start=True, stop=True)
                nc.scalar.activation(out=t2b_t[:, sl], in_=e2p[:, :fw],
                                     func=AF.Tanh, bias=hb_t[:, 1:2])

            # pass 2: l2norms + projection MLPs (sqrt activation set).
            # 1/max(||x||, eps) == 1/sqrt(||x||^2 + eps^2) via the Sqrt bias.
            def inv_norm(x_ap, fw):
                # all-fp16 chain: TensorTensor only has a 2x mode and only
                # for pure 2-byte operands, so keep everything fp16
                sq16 = scp.tile([128, 512], bf16, tag="sqb", bufs=4)
                nc.vector.tensor_tensor(out=sq16[:, :fw], in0=x_ap,
                                        in1=x_ap, op=OP.mult)
                nsq = psA.tile([128, 512], f32, space="PSUM", tag="sc")
                nc.tensor.matmul(out=nsq[:, :fw], lhsT=ones_t[:],
                                 rhs=sq16[:, :fw], start=True, stop=True)
                nrm = scp.tile([128, 512], bf16, tag="nrm", bufs=4)
                nc.scalar.activation(out=nrm[:, :fw], in_=nsq[:, :fw],
                                     func=AF.Sqrt, bias=epsn_t[:])
                with nc.allow_low_precision(reason="fp16 1/norm is plenty"):
                    nc.vector.reciprocal(out=nrm[:, :fw], in_=nrm[:, :fw])
                return nrm

            for ft in range(NF):
                fw = min(512, S - ft * 512)
                sl = slice(ft * 512, ft * 512 + fw)
                # e2 branch: l2norm of tanh
                nrm = inv_norm(t2b_t[:, sl], fw)
                e2b = scp.tile([128, 512], bf16, tag="e2b", bufs=4)
                nc.vector.tensor_tensor(out=e2b[:, :fw], in0=t2b_t[:, sl],
                                        in1=nrm[:, :fw], op=OP.mult)
                nc.sync.dma_start(outs[1][:, sl], e2b[:, :fw])

                # p1 branch
                r1p = psB.tile([128, 512], f32, space="PSUM", tag="dense")
                nc.tensor.matmul(out=r1p[:, :fw], lhsT=wh_t[:, 2, :],
                                 rhs=e1b_t[:, sl], start=True, stop=True)
                r1b = scp.tile([128, 512], bf16, tag="r1b", bufs=3)
                nc.scalar.activation(out=r1b[:, :fw], in_=r1p[:, :fw],
                                     func=AF.Relu, bias=hb_t[:, 2:3])
                z1p = psB.tile([128, 512], f32, space="PSUM", tag="dense")
                nc.tensor.matmul(out=z1p[:, :fw], lhsT=wh_t[:, 3, :],
                                 rhs=r1b[:, :fw], start=True, stop=True)
                z1s = scp.tile([128, 512], bf16, tag="z1s", bufs=3)
                nc.vector.tensor_scalar(out=z1s[:, :fw], in0=z1p[:, :fw],
                                        scalar1=hb_t[:, 3:4], scalar2=None,
                                        op0=OP.add)
                nrm1 = inv_norm(z1s[:, :fw], fw)
                p1s = scp.tile([128, 512], bf16, tag="p1s", bufs=3)
                nc.vector.tensor_tensor(out=p1s[:, :fw], in0=z1s[:, :fw],
                                        in1=nrm1[:, :fw], op=OP.mult)
                nc.sync.dma_start(outs[2][:, sl], p1s[:, :fw])

                # p2 branch
                r2p = psB.tile([128, 512], f32, space="PSUM", tag="dense")
                nc.tensor.matmul(out=r2p[:, :fw], lhsT=wh_t[:, 4, :],
                                 rhs=e2b[:, :fw], start=True, stop=True)
                r2b = scp.tile([128, 512], bf16, tag="r2b", bufs=3)
                nc.scalar.activation(out=r2b[:, :fw], in_=r2p[:, :fw],
                                     func=AF.Relu, bias=hb_t[:, 4:5])
                z2p = psB.tile([128, 512], f32, space="PSUM", tag="dense")
                nc.tensor.matmul(out=z2p[:, :fw], lhsT=wh_t[:, 5, :],
                                 rhs=r2b[:, :fw], start=True, stop=True)
                z2s = scp.tile([128, 512], bf16, tag="z2s", bufs=3)
                nc.vector.tensor_scalar(out=z2s[:, :fw], in0=z2p[:, :fw],
                                        scalar1=hb_t[:, 5:6], scalar2=None,
                                        op0=OP.add)
                nrm2 = inv_norm(z2s[:, :fw], fw)
                p2s = scp.tile([128, 512], bf16, tag="p2s", bufs=3)
                nc.vector.tensor_tensor(out=p2s[:, :fw], in0=z2s[:, :fw],
                                        in1=nrm2[:, :fw], op=OP.mult)
                nc.sync.dma_start(outs[3][:, sl], p2s[:, :fw])

    nc.compile()
    return nc


# ---------------------------------------------------------------- entry point

def _run(inputs, trace=False, trace_kwargs=None, nc_out=None):
    x = np.asarray(inputs["x"], np.float32)
    N = x.shape[0]
    assert N % NCORES == 0
    S = N // NCORES

    d0 = _prep_type(inputs["edge_index0"], N, S)
    d1 = _prep_type(inputs["edge_index1"], N, S)
    (dinv0, dinvsq0, sch0, idx0, rel0, coef0, stot0, nch0) = d0
    (dinv1, dinvsq1, sch1, idx1, rel1, coef1, stot1, nch1) = d1

    nc = _build(N, S, sch0, stot0, nch0, sch1, stot1, nch1)
    if nc_out is not None:
        nc_out.append(nc)

    tab = x.astype(np.float16)

    W0 = np.asarray(inputs["W0"], np.float32)
    W1 = np.asarray(inputs["W1"], np.float32)
    wd = np.zeros((L * 2 * 128, D), np.float32)
    for l in range(L):
        wd[(l * 2) * 128:(l * 2 + 1) * 128] = W0[l]
        wd[(l * 2 + 1) * 128:(l * 2 + 2) * 128] = W1[l]
    gb = np.stack([np.asarray(inputs["gamma"], np.float32).T,
                   np.asarray(inputs["beta"], np.float32).T], 0)
    gb = np.concatenate([gb[0], gb[1]], axis=1)  # [128, 2L]
    wh = np.concatenate([np.asarray(inputs[k], np.float32) for k in
                         ("emb1_W", "emb2_W", "ph1_Wa", "ph1_Wb",
                          "ph2_Wa", "ph2_Wb")], 0)
    hb = np.stack([np.asarray(inputs[k], np.float32) for k in
                   ("emb1_b", "emb2_b", "ph1_ba", "ph1_bb",
                    "ph2_ba", "ph2_bb")], 1)

    iota = np.broadcast_to(np.arange(128, dtype=np.float32),
                           (128, 128)).astype(np.float16)
    ones = np.ones((128, 128), np.float16)

    in_maps = []
    for c in range(NCORES):
        sl = slice(c * S, (c + 1) * S)
        in_maps.append({
            "tab_in": tab,
            "xT_in": np.ascontiguousarray(x[sl].T).astype(np.float16),
            "dinvsq0_in": np.ascontiguousarray(
                np.broadcast_to(dinvsq0[sl], (128, S))).astype(np.float16),
            "dinvsq1_in": np.ascontiguousarray(
                np.broadcast_to(dinvsq1[sl], (128, S))).astype(np.float16),
            "idx0_in": idx0[c], "idx1_in": idx1[c],
            "rel0_in": rel0[c], "rel1_in": rel1[c],
            "coef0_in": coef0[c], "coef1_in": coef1[c],
            "wd_in": wd.astype(np.float16),
            "gb_in": gb, "wh_in": wh.astype(np.float16), "hb_in": hb,
            "iota_in": iota, "ones_in": ones,
        })

    res = run_bass_kernel_spmd(nc, in_maps, list(range(NCORES)),
                               trace=trace, **(trace_kwargs or {}))

    full = {}
    for name in ("e1_o", "e2_o", "p1_o", "p2_o"):
        full[name] = np.concatenate(
            [res.results[c][name].T.astype(np.float32)
             for c in range(NCORES)], axis=0)
    return (full["e1_o"], full["e2_o"], full["p1_o"], full["p2_o"]), res


def kernel(**inputs):
    out, _ = _run(inputs)
    return out


# revision 65
# speedup vs baseline: 1.1105x; 1.1105x over previous
"""Trainium2 Bass kernel for nn_ClusterGCN (3-layer 2-edge-type GCN + heads).

Strategy (8 NeuronCores, node-parallel):
  - Nodes sharded contiguously: core c owns rows [c*S, (c+1)*S), S = N/8.
  - ONE replicated node-major fp16 table of h per layer (not per edge type);
    the full GCN edge coefficient dinv[src]*dinv[dst] is folded into the
    one-hot scatter matrix, so a single AllGather per layer boundary
    suffices. Edges are sharded by dst; messages h[src] are fetched with
    GPSIMD dma_gather (int16 indices, lo/hi base split for N > 32768) and
    scatter-added into a feature-major accumulator via one-hot matmuls on
    the PE: out[feat, dst_slot] += msg[edge, feat]^T @ (coef * onehot).
  - g_t = scatter_t + dinv_t^2 * h adds the self-loop, then
    h' = BN(g0 @ W0 + g1 @ W1) with batch stats AllReduced across cores.
  - Next-layer tables are built with a single XBAR dma transpose of the
    normalized h16 and AllGathered. Heads (tanh/relu/l2norm MLPs) run
    node-sharded in two activation-table passes (tanh pass, sqrt pass).
"""
import math
import numpy as np
import ml_dtypes

import concourse.bacc as bacc
import concourse.bass as bass
import concourse.mybir as mybir
import concourse.tile as tile
from concourse.library_config import mlp as mlp_lib
from concourse.bass_utils import run_bass_kernel_spmd

NCORES = 8
D = 128
L = 3
EPS_BN = 1e-5
EPS_NORM = 1e-12
IDX_LIMIT = 32768
PIECE = 1024          # gather slots per dma_gather instruction (ring sized to fit)
SENT_DST = 320.0      # sentinel dst slot (fp16-exact, >= 128)

f32 = mybir.dt.float32
bf16 = mybir.dt.float16  # (fp16 everywhere: 8x finer mantissa than bf16, same HW rates)
i16 = mybir.dt.int16
AF = mybir.ActivationFunctionType
OP = mybir.AluOpType


# ---------------------------------------------------------------- host prep

def _prep_type(edge_index, N, S):
    """Per edge type: degrees + per-core common-shape gather/scatter schedule."""
    src = np.asarray(edge_index[0], np.int64)
    dst = np.asarray(edge_index[1], np.int64)
    deg = np.bincount(dst, minlength=N).astype(np.float64) + 1.0
    dinv = 1.0 / np.sqrt(deg)

    NW = (S + 127) // 128
    HI_BASE = N - IDX_LIMIT  # hi-region table base; rows [HI_BASE, N)
    # src in [0, IDX_LIMIT) reachable from region 0; [HI_BASE, N) from region 1.
    # srcs in the overlap [max(HI_BASE,0), IDX_LIMIT) are flexible - used to
    # round region-0 groups up to full chunks and minimize sentinel padding.
    cores = []
    for c in range(NCORES):
        m = (dst >= c * S) & (dst < (c + 1) * S)
        s_c = src[m]
        d_c = dst[m]
        dl = d_c - c * S
        w = dl // 128
        order = np.lexsort((s_c, dl, w))
        cores.append((s_c[order], dl[order], w[order], d_c[order]))

    if N > IDX_LIMIT:
        K = np.zeros((2, NW), np.int64)
        must_lo = []
        for (s_c, dl, w, _) in cores:
            cnt_lo = np.bincount(w[s_c < HI_BASE], minlength=NW)
            must_lo.append(cnt_lo)
            K[0] = np.maximum(K[0], (cnt_lo + 127) // 128)
        K[0] = np.maximum(K[0], 1)
        core_reg = []
        for ci, (s_c, dl, w, _) in enumerate(cores):
            reg = (s_c >= IDX_LIMIT).astype(np.int64)
            for w_ in range(NW):
                cap = K[0][w_] * 128
                flex = np.flatnonzero((w == w_) & (s_c >= HI_BASE) & (s_c < IDX_LIMIT))
                take = min(max(cap - int(must_lo[ci][w_]), 0), len(flex))
                reg[flex[:take]] = 0
                reg[flex[take:]] = 1
            cnt_hi = np.bincount(w[reg == 1], minlength=NW)
            K[1] = np.maximum(K[1], (cnt_hi + 127) // 128)
            core_reg.append(reg)
        K[1] = np.maximum(K[1], 1)
        cores = [(s_c, dl, w, d_c, core_reg[ci])
                 for ci, (s_c, dl, w, d_c) in enumerate(cores)]
    else:
        K = np.zeros((2, NW), np.int64)
        for (s_c, dl, w, _) in cores:
            cnt = np.bincount(w, minlength=NW)
            K[0] = np.maximum(K[0], (cnt + 127) // 128)
        K[0] = np.maximum(K[0], 1)
        cores = [(s_c, dl, w, d_c, np.zeros(len(s_c), np.int64))
                 for (s_c, dl, w, d_c) in cores]

    schedule = []  # (region, window, nchunks) in slot order
    for r in (0, 1):
        for w_ in range(NW):
            if K[r][w_] > 0:
                schedule.append((r, int(w_), int(K[r][w_])))
    nchunks = sum(k for _, _, k in schedule)
    stot = nchunks * 128

    idx_all = np.zeros((NCORES, max(stot, 128)), np.int64)
    rel_all = np.full((NCORES, max(nchunks, 1) * 128), SENT_DST, np.float64)
    coef_all = np.zeros((NCORES, max(nchunks, 1) * 128), np.float64)
    for ci, (s_c, dl, w, d_c, reg) in enumerate(cores):
        pos = 0
        for (r, w_, k) in schedule:
            m = (reg == r) & (w == w_)
            n = int(m.sum())
            sv = s_c[m]
            idx_all[ci, pos:pos + n] = sv if r == 0 else sv - (N - IDX_LIMIT)
            rel_all[ci, pos:pos + n] = dl[m] - w_ * 128
            coef_all[ci, pos:pos + n] = dinv[sv] * dinv[d_c[m]]
            pos += k * 128

    # wrapped int16 index layout: idxs[p, s] = idx[s*16 + p%16]
    cols = max(stot // 16, 1)
    idx_w = np.zeros((NCORES, 128, cols), np.int16)
    if stot:
        a = idx_all[:, :stot].reshape(NCORES, cols, 16)  # [c, s, j]
        for p in range(128):
            idx_w[:, p, :] = a[:, :, p % 16]
    # dst-slot / coefficient tiles: [p, chunk] = value of edge chunk*128+p
    rel_t = np.ascontiguousarray(
        rel_all[:, :nchunks * 128].reshape(NCORES, nchunks, 128).transpose(0, 2, 1)
    ).astype(np.float32)
    coef_t = np.ascontiguousarray(
        coef_all[:, :nchunks * 128].reshape(NCORES, nchunks, 128).transpose(0, 2, 1)
    ).astype(np.float32)

    dinvsq = (dinv * dinv).astype(np.float32)
    return dinv.astype(np.float32), dinvsq, schedule, idx_w, rel_t, coef_t, stot, nchunks


def _pieces(schedule):
    """Split slot range into gather pieces that do not cross the lo/hi boundary.
    Returns list of (slot_start, slot_count, region)."""
    out = []
    for r in (0, 1):
        lo = sum(k * 128 for (rr, _, k) in schedule if rr < r)
        n = sum(k * 128 for (rr, _, k) in schedule if rr == r)
        p = lo
        while p < lo + n:
            c = min(PIECE, lo + n - p)
            out.append((p, c, r))
            p += c
    return out


# ---------------------------------------------------------------- device build

def _build(N, S, sch0, stot0, nch0, sch1, stot1, nch1):
    NW = (S + 127) // 128
    SP_ = NW * 128           # padded S (multiple of 128) for the XBAR transpose
    NF = (S + 511) // 512    # 512-wide node tiles
    nc = bacc.Bacc("TRN2", target_bir_lowering=False, debug=False,
                   num_devices=NCORES, dynamic_dma_scratch_size=PIECE * 16)

    def din(name, shape, dt):
        return nc.dram_tensor(name, shape, dt, kind="ExternalInput")

    tab_in = din("tab_in", [N, D], bf16)
    xT_in = din("xT_in", [128, S], bf16)
    dinvsq_in = [din("dinvsq0_in", [128, S], bf16), din("dinvsq1_in", [128, S], bf16)]
    idx_in = [din("idx0_in", [128, max(stot0 // 16, 1)], i16),
              din("idx1_in", [128, max(stot1 // 16, 1)], i16)]
    rel_in = [din("rel0_in", [128, max(nch0, 1)], f32),
              din("rel1_in", [128, max(nch1, 1)], f32)]
    coef_in = [din("coef0_in", [128, max(nch0, 1)], f32),
               din("coef1_in", [128, max(nch1, 1)], f32)]
    wd_in = din("wd_in", [L * 2 * 128, D], bf16)
    gb_in = din("gb_in", [128, 2 * L], f32)
    wh_in = din("wh_in", [6 * 128, D], bf16)
    hb_in = din("hb_in", [128, 6], f32)
    iota_in = din("iota_in", [128, 128], bf16)
    ones_in = din("ones_in", [128, 128], bf16)

    outs = [nc.dram_tensor(n, [128, S], bf16, kind="ExternalOutput")
            for n in ("e1_o", "e2_o", "p1_o", "p2_o")]

    with tile.TileContext(nc) as tc:
        with (
            tc.tile_pool(name="const", bufs=1) as const,
            tc.tile_pool(name="g", bufs=1) as gpool,
            tc.tile_pool(name="msg", bufs=8) as msgp,
            tc.tile_pool(name="oh", bufs=64) as ohp,
            tc.tile_pool(name="scr", bufs=1) as scp,
            tc.tile_pool(name="psA", bufs=4, space="PSUM") as psA,
            tc.tile_pool(name="psB", bufs=4, space="PSUM") as psB,
            tc.tile_pool(name="dram", bufs=1, space="DRAM") as dram,
        ):
            nc.gpsimd.load_library(mlp_lib)

            # ---- persistent SBUF tiles
            iota_t = const.tile([128, 128], bf16)
            ones_t = const.tile([128, 128], bf16)
            dinvsq_t = [const.tile([128, S], bf16, tag=f"dq{t}", name=f"dq{t}")
                        for t in (0, 1)]
            idx_t = [const.tile([128, max(stot0 // 16, 1)], i16, tag="idx0", name="idx0"),
                     const.tile([128, max(stot1 // 16, 1)], i16, tag="idx1", name="idx1")]
            rel_t = [const.tile([128, max(nch0, 1)], f32, tag="rel0", name="rel0"),
                     const.tile([128, max(nch1, 1)], f32, tag="rel1", name="rel1")]
            coef_t = [const.tile([128, max(nch0, 1)], f32, tag="coef0", name="coef0"),
                      const.tile([128, max(nch1, 1)], f32, tag="coef1", name="coef1")]
            wd_t = const.tile([128, L * 2, D], bf16)     # dense weights
            wh_t = const.tile([128, 6, D], bf16)         # head weights
            gb_t = const.tile([128, 2 * L], f32)
            hb_t = const.tile([128, 6], f32)
            epsn_t = const.tile([128, 1], f32)
            nc.vector.memset(epsn_t[:], EPS_NORM * EPS_NORM)

            h16_t = gpool.tile([128, SP_], bf16, tag="h16")      # padded cols
            h_pre = gpool.tile([128, S], f32, tag="hpre")
            gbf_t = [gpool.tile([128, S], bf16, tag=f"gbf{t}", name=f"gbf{t}")
                     for t in (0, 1)]
            # stage (layer boundaries) and e1b (heads) never live at the same
            # time; alias them in one padded buffer to save SBUF.
            ub = gpool.tile([128, SP_], bf16, tag="stage")
            stage = ub[:].rearrange("p (w k) -> p w k", k=128)
            e1b_t = ub[:]
            t2b_t = gpool.tile([128, S], bf16, tag="t2b")
            # region-1 scatter accumulators, aliased over boundary/heads-only
            # buffers (stage/ub and t2b are idle during scatter+dense)
            grf_t = [ub[:, :S], t2b_t[:]]

            nc.sync.dma_start(iota_t[:], iota_in[:])
            nc.sync.dma_start(ones_t[:], ones_in[:])
            for t in (0, 1):
                nc.sync.dma_start(dinvsq_t[t][:], dinvsq_in[t][:])
                nc.sync.dma_start(idx_t[t][:], idx_in[t][:])
                nc.sync.dma_start(rel_t[t][:], rel_in[t][:])
                nc.sync.dma_start(coef_t[t][:], coef_in[t][:])
            nc.sync.dma_start(
                wd_t[:], wd_in[:].rearrange("(k p) d -> p k d", p=128))
            nc.sync.dma_start(
                wh_t[:], wh_in[:].rearrange("(k p) d -> p k d", p=128))
            nc.sync.dma_start(gb_t[:], gb_in[:])
            nc.sync.dma_start(hb_t[:], hb_in[:])
            nc.sync.dma_start(h16_t[:, :S], xT_in[:])
            if SP_ > S:
                nc.vector.memset(h16_t[:, S:], 0.0)

            # ---- internal DRAM for collectives
            ag_in = {}
            ag_out = {}
            for l in (0, 1):
                ag_in[l] = dram.tile([S, D], bf16, tag=f"agi{l}", name=f"agi{l}")
                ag_out[l] = dram.tile([N, D], bf16, addr_space="Shared",
                                      tag=f"ago{l}", name=f"ago{l}")
            st_in = [dram.tile([128, 2], f32, tag=f"sti{l}", name=f"sti{l}")
                     for l in range(L)]
            st_out = [dram.tile([128, 2], f32, addr_space="Shared",
                                tag=f"sto{l}", name=f"sto{l}") for l in range(L)]

            schs = (sch0, sch1)
            has_r1 = any(r == 1 for (r, _, _) in sch0)
            rg = [list(range(NCORES))]

            for l in range(L):
                # ---------------- scatter phase (both edge types)
                for t in (0, 1):
                    if l == 0:
                        tab_lo = tab_in[:]
                        tab_hi = tab_in[N - IDX_LIMIT:] if N > IDX_LIMIT else None
                    else:
                        tab_lo = ag_out[l - 1][:]
                        tab_hi = ag_out[l - 1][N - IDX_LIMIT:] \
                            if N > IDX_LIMIT else None

                    sch = schs[t]
                    # chunk meta: (region, window, win_first, win_last,
                    #              group_first, group_last); groups = 4 windows
                    # of one region sharing a [128,512] PSUM bank.
                    chunk_meta = []
                    for si, (r, w_, k) in enumerate(sch):
                        gf = (w_ % 4 == 0) or si == 0 or sch[si - 1][0] != r
                        gl = (w_ % 4 == 3) or si == len(sch) - 1 \
                            or sch[si + 1][0] != r
                        for j in range(k):
                            chunk_meta.append(
                                (r, w_, j == 0, j == k - 1,
                                 gf and j == 0, gl and j == k - 1))

                    pieces = _pieces(sch)
                    acc = None
                    for (p0, cnt, r) in pieces:
                        msg = msgp.tile([128, PIECE // 128, 128], bf16,
                                        tag="msg")
                        src_ap = tab_lo if r == 0 else tab_hi
                        nc.gpsimd.dma_gather(
                            msg[:, :cnt // 128, :], src_ap,
                            idx_t[t][:, p0 // 16:(p0 + cnt) // 16],
                            num_idxs=cnt, num_idxs_reg=cnt, elem_size=D,
                        )
                        for ci in range(cnt // 128):
                            gc = p0 // 128 + ci
                            (cr, w_, first, last, gfirst, glast) = chunk_meta[gc]
                            oh = ohp.tile([128, 128], bf16, tag="oh")
                            nc.vector.tensor_scalar(
                                out=oh[:], in0=iota_t[:],
                                scalar1=rel_t[t][:, gc:gc + 1],
                                scalar2=coef_t[t][:, gc:gc + 1],
                                op0=OP.is_equal, op1=OP.mult,
                            )
                            if gfirst:
                                acc = psA.tile([128, 512], f32, space="PSUM",
                                               tag="sc")
                            ws = (w_ % 4) * 128
                            nc.tensor.matmul(out=acc[:, ws:ws + 128],
                                             lhsT=msg[:, ci, :],
                                             rhs=oh[:], start=first, stop=last)
                            if glast:
                                base = (w_ // 4) * 512
                                wd = min(512, S - base)
                                # evacuate scatter PSUM on Act; each region
                                # gets its own fp16 accumulator and the dense
                                # phase sums them (extra matmul pair)
                                dst = gbf_t[t] if cr == 0 else grf_t[t]
                                nc.scalar.activation(
                                    out=dst[:, base:base + wd],
                                    in_=acc[:, :wd], func=AF.Identity)

                # ---------------- dense + stats partials
                sum_p = scp.tile([128, NF], f32, tag="sump")
                ssq_p = scp.tile([128, NF], f32, tag="ssqp")
                for ft in range(NF):
                    fw = min(512, S - ft * 512)
                    sl = slice(ft * 512, ft * 512 + fw)
                    # self-loop tiles: sl_t = dinv^2 * h  (fp16, 4x DVE)
                    slt = [scp.tile([128, 512], bf16, tag=f"sl{t}", bufs=3,
                                    name=f"sl{t}") for t in (0, 1)]
                    for t in (0, 1):
                        nc.vector.tensor_tensor(out=slt[t][:, :fw],
                                                in0=h16_t[:, sl],
                                                in1=dinvsq_t[t][:, sl],
                                                op=OP.mult)
                    dp = psB.tile([128, 512], f32, space="PSUM", tag="dense")
                    rhss = [(0, gbf_t[0][:, sl]), (1, gbf_t[1][:, sl]),
                            (0, slt[0][:, :fw]), (1, slt[1][:, :fw])]
                    if has_r1:
                        rhss += [(0, grf_t[0][:, sl]), (1, grf_t[1][:, sl])]
                    for mi, (t, rhs_ap) in enumerate(rhss):
                        nc.tensor.matmul(out=dp[:, :fw],
                                         lhsT=wd_t[:, l * 2 + t, :],
                                         rhs=rhs_ap, start=(mi == 0),
                                         stop=(mi == len(rhss) - 1))
                    nc.scalar.activation(out=h_pre[:, sl], in_=dp[:, :fw],
                                         func=AF.Identity,
                                         accum_out=sum_p[:, ft:ft + 1])
                    sq = scp.tile([128, 512], bf16, tag="sq", bufs=3)
                    nc.scalar.activation(out=sq[:, :fw], in_=dp[:, :fw],
                                         func=AF.Square,
                                         accum_out=ssq_p[:, ft:ft + 1])

                # ---------------- BN stats allreduce
                st = scp.tile([128, 2], f32, tag="st")
                nc.vector.tensor_reduce(out=st[:, 0:1], in_=sum_p[:],
                                        axis=mybir.AxisListType.X, op=OP.add)
                nc.vector.tensor_reduce(out=st[:, 1:2], in_=ssq_p[:],
                                        axis=mybir.AxisListType.X, op=OP.add)
                nc.sync.dma_start(st_in[l][:], st[:])
                nc.gpsimd.collective_compute(
                    "AllReduce", OP.add, replica_groups=rg,
                    ins=[st_in[l].opt()], outs=[st_out[l].opt()])
                sta = scp.tile([128, 2], f32, tag="sta")
                nc.sync.dma_start(sta[:], st_out[l][:])

                mean = scp.tile([128, 1], f32, tag="mean")
                var = scp.tile([128, 1], f32, tag="var")
                scl = scp.tile([128, 1], f32, tag="scl")
                sht = scp.tile([128, 1], f32, tag="sht")
                tmp = scp.tile([128, 1], f32, tag="tmp1")
                inv_n = 1.0 / float(N)
                nc.vector.tensor_scalar(out=mean[:], in0=sta[:, 0:1],
                                        scalar1=inv_n, scalar2=None, op0=OP.mult)
                nc.vector.tensor_scalar(out=var[:], in0=sta[:, 1:2],
                                        scalar1=inv_n, scalar2=None, op0=OP.mult)
                nc.vector.tensor_tensor(out=tmp[:], in0=mean[:], in1=mean[:],
                                        op=OP.mult)
                nc.vector.tensor_tensor(out=var[:], in0=var[:], in1=tmp[:],
                                        op=OP.subtract)
                # scl = gamma / sqrt(var + eps); sht = beta - mean*scl
                nc.vector.tensor_scalar(out=var[:], in0=var[:], scalar1=EPS_BN,
                                        scalar2=None, op0=OP.add)
                nc.scalar.activation(out=tmp[:], in_=var[:], func=AF.Sqrt)
                nc.vector.reciprocal(out=tmp[:], in_=tmp[:])
                nc.vector.tensor_tensor(out=scl[:], in0=gb_t[:, l:l + 1],
                                        in1=tmp[:], op=OP.mult)
                nc.vector.tensor_tensor(out=tmp[:], in0=mean[:], in1=scl[:],
                                        op=OP.mult)
                nc.vector.tensor_tensor(out=sht[:], in0=gb_t[:, L + l:L + l + 1],
                                        in1=tmp[:], op=OP.subtract)

                # ---------------- normalize (+ relu except last layer)
                if l < L - 1:
                    nc.scalar.activation(out=h16_t[:, :S], in_=h_pre[:],
                                         func=AF.Relu,
                                         bias=sht[:], scale=scl[:])
                else:
                    # tiled so the heads pipeline behind the normalize
                    for ft in range(NF):
                        fw = min(512, S - ft * 512)
                        sl = slice(ft * 512, ft * 512 + fw)
                        nc.scalar.activation(out=h16_t[:, sl],
                                             in_=h_pre[:, sl],
                                             func=AF.Identity,
                                             bias=sht[:], scale=scl[:])

                # ---------------- next-layer table: XBAR transpose + allgather
                if l < L - 1:
                    nc.sync.dma_start_transpose(stage[:], h16_t[:])
                    full = (S // 128) * 128
                    nc.sync.dma_start(
                        ag_in[l][:full].rearrange("(w p) d -> p w d", p=128),
                        stage[:, :S // 128, :])
                    if S > full:
                        nc.sync.dma_start(ag_in[l][full:],
                                          stage[:S - full, S // 128, :])
                    nc.gpsimd.collective_compute(
                        "AllGather", OP.bypass, replica_groups=rg,
                        ins=[ag_in[l].opt()], outs=[ag_out[l].opt()])

            # ---------------- heads (two activation-table passes)
            # pass 1: tanh embeddings, written straight to fp16 buffers
            for ft in range(NF):
                fw = min(512, S - ft * 512)
                sl = slice(ft * 512, ft * 512 + fw)
                e1p = psB.tile([128, 512], f32, space="PSUM", tag="dense")
                nc.tensor.matmul(out=e1p[:, :fw], lhsT=wh_t[:, 0, :],
                                 rhs=h16_t[:, sl], start=True, stop=True)
                nc.scalar.activation(out=e1b_t[:, sl], in_=e1p[:, :fw],
                                     func=AF.Tanh, bias=hb_t[:, 0:1])
                nc.sync.dma_start(outs[0][:, sl], e1b_t[:, sl])
                e2p = psB.tile([128, 512], f32, space="PSUM", tag="dense")
                nc.tensor.matmul(out=e2p[:, :fw], lhsT=wh_t[:, 1, :],
                                 rhs=h16_t[:, sl], start=True, stop=True)
                nc.scalar.activation(out=t2b_t[:, sl], in_=e2p[:, :fw],
                                     func=AF.Tanh, bias=hb_t[:, 1:2])

            # pass 2: l2norms + projection MLPs (sqrt activation set).
            # 1/max(||x||, eps) == 1/sqrt(||x||^2 + eps^2) via the Sqrt bias.
            def inv_norm(x_ap, fw):
                # all-fp16 chain: TensorTensor only has a 2x mode and only
                # for pure 2-byte operands, so keep everything fp16
                sq16 = scp.tile([128, 512], bf16, tag="sqb", bufs=4)
                nc.vector.tensor_tensor(out=sq16[:, :fw], in0=x_ap,
                                        in1=x_ap, op=OP.mult)
                nsq = psA.tile([128, 512], f32, space="PSUM", tag="sc")
                nc.tensor.matmul(out=nsq[:, :fw], lhsT=ones_t[:],
                                 rhs=sq16[:, :fw], start=True, stop=True)
                nrm = scp.tile([128, 512], bf16, tag="nrm", bufs=4)
                nc.scalar.activation(out=nrm[:, :fw], in_=nsq[:, :fw],
                                     func=AF.Sqrt, bias=epsn_t[:])
                with nc.allow_low_precision(reason="fp16 1/norm is plenty"):
                    nc.vector.reciprocal(out=nrm[:, :fw], in_=nrm[:, :fw])
                return nrm

            # pass 2a: e2 = l2norm(tanh) scaled in place in t2b
            for ft in range(NF):
                fw = min(512, S - ft * 512)
                sl = slice(ft * 512, ft * 512 + fw)
                nrm = inv_norm(t2b_t[:, sl], fw)
                nc.vector.tensor_tensor(out=t2b_t[:, sl], in0=t2b_t[:, sl],
                                        in1=nrm[:, :fw], op=OP.mult)
                nc.sync.dma_start(outs[1][:, sl], t2b_t[:, sl])

            # pass 2b: p1 projection (2 PSUM banks/tile -> 2-tile overlap)
            for ft in range(NF):
                fw = min(512, S - ft * 512)
                sl = slice(ft * 512, ft * 512 + fw)
                r1p = psB.tile([128, 512], f32, space="PSUM", tag="dense")
                nc.tensor.matmul(out=r1p[:, :fw], lhsT=wh_t[:, 2, :],
                                 rhs=e1b_t[:, sl], start=True, stop=True)
                r1b = scp.tile([128, 512], bf16, tag="r1b", bufs=3)
                nc.scalar.activation(out=r1b[:, :fw], in_=r1p[:, :fw],
                                     func=AF.Relu, bias=hb_t[:, 2:3])
                z1p = psB.tile([128, 512], f32, space="PSUM", tag="dense")
                nc.tensor.matmul(out=z1p[:, :fw], lhsT=wh_t[:, 3, :],
                                 rhs=r1b[:, :fw], start=True, stop=True)
                z1s = scp.tile([128, 512], bf16, tag="z1s", bufs=3)
                nc.vector.tensor_scalar(out=z1s[:, :fw], in0=z1p[:, :fw],
                                        scalar1=hb_t[:, 3:4], scalar2=None,
                                        op0=OP.add)
                nrm1 = inv_norm(z1s[:, :fw], fw)
                p1s = scp.tile([128, 512], bf16, tag="p1s", bufs=3)
                nc.vector.tensor_tensor(out=p1s[:, :fw], in0=z1s[:, :fw],
                                        in1=nrm1[:, :fw], op=OP.mult)
                nc.sync.dma_start(outs[2][:, sl], p1s[:, :fw])

            # pass 2c: p2 projection from the in-place e2 in t2b
            for ft in range(NF):
                fw = min(512, S - ft * 512)
                sl = slice(ft * 512, ft * 512 + fw)
                r2p = psB.tile([128, 512], f32, space="PSUM", tag="dense")
                nc.tensor.matmul(out=r2p[:, :fw], lhsT=wh_t[:, 4, :],
                                 rhs=t2b_t[:, sl], start=True, stop=True)
                r2b = scp.tile([128, 512], bf16, tag="r2b", bufs=3)
                nc.scalar.activation(out=r2b[:, :fw], in_=r2p[:, :fw],
                                     func=AF.Relu, bias=hb_t[:, 4:5])
                z2p = psB.tile([128, 512], f32, space="PSUM", tag="dense")
                nc.tensor.matmul(out=z2p[:, :fw], lhsT=wh_t[:, 5, :],
                                 rhs=r2b[:, :fw], start=True, stop=True)
                z2s = scp.tile([128, 512], bf16, tag="z2s", bufs=3)
                nc.vector.tensor_scalar(out=z2s[:, :fw], in0=z2p[:, :fw],
                                        scalar1=hb_t[:, 5:6], scalar2=None,
                                        op0=OP.add)
                nrm2 = inv_norm(z2s[:, :fw], fw)
                p2s = scp.tile([128, 512], bf16, tag="p2s", bufs=3)
                nc.vector.tensor_tensor(out=p2s[:, :fw], in0=z2s[:, :fw],
                                        in1=nrm2[:, :fw], op=OP.mult)
                nc.sync.dma_start(outs[3][:, sl], p2s[:, :fw])

    nc.compile()
    return nc


# ---------------------------------------------------------------- entry point

def _run(inputs, trace=False, trace_kwargs=None, nc_out=None):
    x = np.asarray(inputs["x"], np.float32)
    N = x.shape[0]
    assert N % NCORES == 0
    S = N // NCORES

    d0 = _prep_type(inputs["edge_index0"], N, S)
    d1 = _prep_type(inputs["edge_index1"], N, S)
    (dinv0, dinvsq0, sch0, idx0, rel0, coef0, stot0, nch0) = d0
    (dinv1, dinvsq1, sch1, idx1, rel1, coef1, stot1, nch1) = d1

    nc = _build(N, S, sch0, stot0, nch0, sch1, stot1, nch1)
    if nc_out is not None:
        nc_out.append(nc)

    tab = x.astype(np.float16)

    W0 = np.asarray(inputs["W0"], np.float32)
    W1 = np.asarray(inputs["W1"], np.float32)
    wd = np.zeros((L * 2 * 128, D), np.float32)
    for l in range(L):
        wd[(l * 2) * 128:(l * 2 + 1) * 128] = W0[l]
        wd[(l * 2 + 1) * 128:(l * 2 + 2) * 128] = W1[l]
    gb = np.stack([np.asarray(inputs["gamma"], np.float32).T,
                   np.asarray(inputs["beta"], np.float32).T], 0)
    gb = np.concatenate([gb[0], gb[1]], axis=1)  # [128, 2L]
    wh = np.concatenate([np.asarray(inputs[k], np.float32) for k in
                         ("emb1_W", "emb2_W", "ph1_Wa", "ph1_Wb",
                          "ph2_Wa", "ph2_Wb")], 0)
    hb = np.stack([np.asarray(inputs[k], np.float32) for k in
                   ("emb1_b", "emb2_b", "ph1_ba", "ph1_bb",
                    "ph2_ba", "ph2_bb")], 1)

    iota = np.broadcast_to(np.arange(128, dtype=np.float32),
                           (128, 128)).astype(np.float16)
    ones = np.ones((128, 128), np.float16)

    in_maps = []
    for c in range(NCORES):
        sl = slice(c * S, (c + 1) * S)
        in_maps.append({
            "tab_in": tab,
            "xT_in": np.ascontiguousarray(x[sl].T).astype(np.float16),
            "dinvsq0_in": np.ascontiguousarray(
                np.broadcast_to(dinvsq0[sl], (128, S))).astype(np.float16),
            "dinvsq1_in": np.ascontiguousarray(
                np.broadcast_to(dinvsq1[sl], (128, S))).astype(np.float16),
            "idx0_in": idx0[c], "idx1_in": idx1[c],
            "rel0_in": rel0[c], "rel1_in": rel1[c],
            "coef0_in": coef0[c], "coef1_in": coef1[c],
            "wd_in": wd.astype(np.float16),
            "gb_in": gb, "wh_in": wh.astype(np.float16), "hb_in": hb,
            "iota_in": iota, "ones_in": ones,
        })

    res = run_bass_kernel_spmd(nc, in_maps, list(range(NCORES)),
                               trace=trace, **(trace_kwargs or {}))

    full = {}
    for name in ("e1_o", "e2_o", "p1_o", "p2_o"):
        full[name] = np.concatenate(
            [res.results[c][name].T.astype(np.float32)
             for c in range(NCORES)], axis=0)
    return (full["e1_o"], full["e2_o"], full["p1_o"], full["p2_o"]), res


def kernel(**inputs):
    out, _ = _run(inputs)
    return out


# revision 67
# speedup vs baseline: 1.1230x; 1.0112x over previous
"""Trainium2 Bass kernel for nn_ClusterGCN (3-layer 2-edge-type GCN + heads).

Strategy (8 NeuronCores, node-parallel):
  - Nodes sharded contiguously: core c owns rows [c*S, (c+1)*S), S = N/8.
  - ONE replicated node-major fp16 table of h per layer (not per edge type);
    the full GCN edge coefficient dinv[src]*dinv[dst] is folded into the
    one-hot scatter matrix, so a single AllGather per layer boundary
    suffices. Edges are sharded by dst; messages h[src] are fetched with
    GPSIMD dma_gather (int16 indices, lo/hi base split for N > 32768) and
    scatter-added into a feature-major accumulator via one-hot matmuls on
    the PE: out[feat, dst_slot] += msg[edge, feat]^T @ (coef * onehot).
  - g_t = scatter_t + dinv_t^2 * h adds the self-loop, then
    h' = BN(g0 @ W0 + g1 @ W1) with batch stats AllReduced across cores.
  - Next-layer tables are built with a single XBAR dma transpose of the
    normalized h16 and AllGathered. Heads (tanh/relu/l2norm MLPs) run
    node-sharded in two activation-table passes (tanh pass, sqrt pass).
"""
import math
import numpy as np
import ml_dtypes

import concourse.bacc as bacc
import concourse.bass as bass
import concourse.mybir as mybir
import concourse.tile as tile
from concourse.library_config import mlp as mlp_lib
from concourse.bass_utils import run_bass_kernel_spmd

NCORES = 8
D = 128
L = 3
EPS_BN = 1e-5
EPS_NORM = 1e-12
IDX_LIMIT = 32768
PIECE = 1024          # gather slots per dma_gather instruction (ring sized to fit)
SENT_DST = 320.0      # sentinel dst slot (fp16-exact, >= 128)

f32 = mybir.dt.float32
bf16 = mybir.dt.float16  # (fp16 everywhere: 8x finer mantissa than bf16, same HW rates)
i16 = mybir.dt.int16
AF = mybir.ActivationFunctionType
OP = mybir.AluOpType


# ---------------------------------------------------------------- host prep

def _prep_type(edge_index, N, S):
    """Per edge type: degrees + per-core common-shape gather/scatter schedule."""
    src = np.asarray(edge_index[0], np.int64)
    dst = np.asarray(edge_index[1], np.int64)
    deg = np.bincount(dst, minlength=N).astype(np.float64) + 1.0
    dinv = 1.0 / np.sqrt(deg)

    NW = (S + 127) // 128
    HI_BASE = N - IDX_LIMIT  # hi-region table base; rows [HI_BASE, N)
    # src in [0, IDX_LIMIT) reachable from region 0; [HI_BASE, N) from region 1.
    # srcs in the overlap [max(HI_BASE,0), IDX_LIMIT) are flexible - used to
    # round region-0 groups up to full chunks and minimize sentinel padding.
    cores = []
    for c in range(NCORES):
        m = (dst >= c * S) & (dst < (c + 1) * S)
        s_c = src[m]
        d_c = dst[m]
        dl = d_c - c * S
        w = dl // 128
        order = np.lexsort((s_c, dl, w))
        cores.append((s_c[order], dl[order], w[order], d_c[order]))

    if N > IDX_LIMIT:
        K = np.zeros((2, NW), np.int64)
        must_lo = []
        for (s_c, dl, w, _) in cores:
            cnt_lo = np.bincount(w[s_c < HI_BASE], minlength=NW)
            must_lo.append(cnt_lo)
            K[0] = np.maximum(K[0], (cnt_lo + 127) // 128)
        K[0] = np.maximum(K[0], 1)
        core_reg = []
        for ci, (s_c, dl, w, _) in enumerate(cores):
            reg = (s_c >= IDX_LIMIT).astype(np.int64)
            for w_ in range(NW):
                cap = K[0][w_] * 128
                flex = np.flatnonzero((w == w_) & (s_c >= HI_BASE) & (s_c < IDX_LIMIT))
                take = min(max(cap - int(must_lo[ci][w_]), 0), len(flex))
                reg[flex[:take]] = 0
                reg[flex[take:]] = 1
            cnt_hi = np.bincount(w[reg == 1], minlength=NW)
            K[1] = np.maximum(K[1], (cnt_hi + 127) // 128)
            core_reg.append(reg)
        K[1] = np.maximum(K[1], 1)
        cores = [(s_c, dl, w, d_c, core_reg[ci])
                 for ci, (s_c, dl, w, d_c) in enumerate(cores)]
    else:
        K = np.zeros((2, NW), np.int64)
        for (s_c, dl, w, _) in cores:
            cnt = np.bincount(w, minlength=NW)
            K[0] = np.maximum(K[0], (cnt + 127) // 128)
        K[0] = np.maximum(K[0], 1)
        cores = [(s_c, dl, w, d_c, np.zeros(len(s_c), np.int64))
                 for (s_c, dl, w, d_c) in cores]

    schedule = []  # (region, window, nchunks) in slot order
    for r in (0, 1):
        for w_ in range(NW):
            if K[r][w_] > 0:
                schedule.append((r, int(w_), int(K[r][w_])))
    nchunks = sum(k for _, _, k in schedule)
    stot = nchunks * 128

    idx_all = np.zeros((NCORES, max(stot, 128)), np.int64)
    rel_all = np.full((NCORES, max(nchunks, 1) * 128), SENT_DST, np.float64)
    coef_all = np.zeros((NCORES, max(nchunks, 1) * 128), np.float64)
    for ci, (s_c, dl, w, d_c, reg) in enumerate(cores):
        pos = 0
        for (r, w_, k) in schedule:
            m = (reg == r) & (w == w_)
            n = int(m.sum())
            sv = s_c[m]
            idx_all[ci, pos:pos + n] = sv if r == 0 else sv - (N - IDX_LIMIT)
            rel_all[ci, pos:pos + n] = dl[m] - w_ * 128
            coef_all[ci, pos:pos + n] = dinv[sv] * dinv[d_c[m]]
            pos += k * 128

    # wrapped int16 index layout: idxs[p, s] = idx[s*16 + p%16]
    cols = max(stot // 16, 1)
    idx_w = np.zeros((NCORES, 128, cols), np.int16)
    if stot:
        a = idx_all[:, :stot].reshape(NCORES, cols, 16)  # [c, s, j]
        for p in range(128):
            idx_w[:, p, :] = a[:, :, p % 16]
    # dst-slot / coefficient tiles: [p, chunk] = value of edge chunk*128+p
    rel_t = np.ascontiguousarray(
        rel_all[:, :nchunks * 128].reshape(NCORES, nchunks, 128).transpose(0, 2, 1)
    ).astype(np.float32)
    coef_t = np.ascontiguousarray(
        coef_all[:, :nchunks * 128].reshape(NCORES, nchunks, 128).transpose(0, 2, 1)
    ).astype(np.float32)

    dinvsq = (dinv * dinv).astype(np.float32)
    return dinv.astype(np.float32), dinvsq, schedule, idx_w, rel_t, coef_t, stot, nchunks


def _pieces(schedule):
    """Split slot range into gather pieces that do not cross the lo/hi boundary.
    Returns list of (slot_start, slot_count, region)."""
    out = []
    for r in (0, 1):
        lo = sum(k * 128 for (rr, _, k) in schedule if rr < r)
        n = sum(k * 128 for (rr, _, k) in schedule if rr == r)
        p = lo
        while p < lo + n:
            c = min(PIECE, lo + n - p)
            out.append((p, c, r))
            p += c
    return out


# ---------------------------------------------------------------- device build

def _build(N, S, sch0, stot0, nch0, sch1, stot1, nch1):
    NW = (S + 127) // 128
    SP_ = NW * 128           # padded S (multiple of 128) for the XBAR transpose
    NF = (S + 511) // 512    # 512-wide node tiles
    nc = bacc.Bacc("TRN2", target_bir_lowering=False, debug=False,
                   num_devices=NCORES, dynamic_dma_scratch_size=PIECE * 16)

    def din(name, shape, dt):
        return nc.dram_tensor(name, shape, dt, kind="ExternalInput")

    tab_in = din("tab_in", [N, D], bf16)
    xT_in = din("xT_in", [128, S], bf16)
    dinvsq_in = [din("dinvsq0_in", [128, S], bf16), din("dinvsq1_in", [128, S], bf16)]
    idx_in = [din("idx0_in", [128, max(stot0 // 16, 1)], i16),
              din("idx1_in", [128, max(stot1 // 16, 1)], i16)]
    rel_in = [din("rel0_in", [128, max(nch0, 1)], f32),
              din("rel1_in", [128, max(nch1, 1)], f32)]
    coef_in = [din("coef0_in", [128, max(nch0, 1)], f32),
               din("coef1_in", [128, max(nch1, 1)], f32)]
    wd_in = din("wd_in", [L * 2 * 128, D], bf16)
    gb_in = din("gb_in", [128, 2 * L], f32)
    wh_in = din("wh_in", [6 * 128, D], bf16)
    hb_in = din("hb_in", [128, 6], f32)
    iota_in = din("iota_in", [128, 128], bf16)
    ones_in = din("ones_in", [128, 128], bf16)

    outs = [nc.dram_tensor(n, [128, S], bf16, kind="ExternalOutput")
            for n in ("e1_o", "e2_o", "p1_o", "p2_o")]

    with tile.TileContext(nc) as tc:
        with (
            tc.tile_pool(name="const", bufs=1) as const,
            tc.tile_pool(name="g", bufs=1) as gpool,
            tc.tile_pool(name="msg", bufs=8) as msgp,
            tc.tile_pool(name="oh", bufs=64) as ohp,
            tc.tile_pool(name="scr", bufs=1) as scp,
            tc.tile_pool(name="psA", bufs=4, space="PSUM") as psA,
            tc.tile_pool(name="psB", bufs=4, space="PSUM") as psB,
            tc.tile_pool(name="dram", bufs=1, space="DRAM") as dram,
        ):
            nc.gpsimd.load_library(mlp_lib)

            # ---- persistent SBUF tiles
            iota_t = const.tile([128, 128], bf16)
            ones_t = const.tile([128, 128], bf16)
            dinvsq_t = [const.tile([128, S], bf16, tag=f"dq{t}", name=f"dq{t}")
                        for t in (0, 1)]
            idx_t = [const.tile([128, max(stot0 // 16, 1)], i16, tag="idx0", name="idx0"),
                     const.tile([128, max(stot1 // 16, 1)], i16, tag="idx1", name="idx1")]
            rel_t = [const.tile([128, max(nch0, 1)], f32, tag="rel0", name="rel0"),
                     const.tile([128, max(nch1, 1)], f32, tag="rel1", name="rel1")]
            coef_t = [const.tile([128, max(nch0, 1)], f32, tag="coef0", name="coef0"),
                      const.tile([128, max(nch1, 1)], f32, tag="coef1", name="coef1")]
            wd_t = const.tile([128, L * 2, D], bf16)     # dense weights
            wh_t = const.tile([128, 6, D], bf16)         # head weights
            gb_t = const.tile([128, 2 * L], f32)
            hb_t = const.tile([128, 6], f32)
            epsn_t = const.tile([128, 1], f32)
            nc.vector.memset(epsn_t[:], EPS_NORM * EPS_NORM)

            h16_t = gpool.tile([128, SP_], bf16, tag="h16")      # padded cols
            h_pre = gpool.tile([128, S], f32, tag="hpre")
            gbf_t = [gpool.tile([128, S], bf16, tag=f"gbf{t}", name=f"gbf{t}")
                     for t in (0, 1)]
            # stage (layer boundaries) and e1b (heads) never live at the same
            # time; alias them in one padded buffer to save SBUF.
            ub = gpool.tile([128, SP_], bf16, tag="stage")
            stage = ub[:].rearrange("p (w k) -> p w k", k=128)
            e1b_t = ub[:]
            t2b_t = gpool.tile([128, S], bf16, tag="t2b")
            # region-1 scatter accumulators, aliased over boundary/heads-only
            # buffers (stage/ub and t2b are idle during scatter+dense)
            grf_t = [ub[:, :S], t2b_t[:]]

            nc.sync.dma_start(iota_t[:], iota_in[:])
            nc.sync.dma_start(ones_t[:], ones_in[:])
            for t in (0, 1):
                nc.sync.dma_start(dinvsq_t[t][:], dinvsq_in[t][:])
                nc.sync.dma_start(idx_t[t][:], idx_in[t][:])
                nc.sync.dma_start(rel_t[t][:], rel_in[t][:])
                nc.sync.dma_start(coef_t[t][:], coef_in[t][:])
            nc.sync.dma_start(
                wd_t[:], wd_in[:].rearrange("(k p) d -> p k d", p=128))
            nc.sync.dma_start(
                wh_t[:], wh_in[:].rearrange("(k p) d -> p k d", p=128))
            nc.sync.dma_start(gb_t[:], gb_in[:])
            nc.sync.dma_start(hb_t[:], hb_in[:])
            nc.sync.dma_start(h16_t[:, :S], xT_in[:])
            if SP_ > S:
                nc.vector.memset(h16_t[:, S:], 0.0)

            # ---- internal DRAM for collectives
            ag_in = {}
            ag_out = {}
            for l in (0, 1):
                ag_in[l] = dram.tile([S, D], bf16, tag=f"agi{l}", name=f"agi{l}")
                ag_out[l] = dram.tile([N, D], bf16, addr_space="Shared",
                                      tag=f"ago{l}", name=f"ago{l}")
            st_in = [dram.tile([128, 2], f32, tag=f"sti{l}", name=f"sti{l}")
                     for l in range(L)]
            st_out = [dram.tile([128, 2], f32, addr_space="Shared",
                                tag=f"sto{l}", name=f"sto{l}") for l in range(L)]

            schs = (sch0, sch1)
            has_r1 = any(r == 1 for (r, _, _) in sch0)
            rg = [list(range(NCORES))]

            for l in range(L):
                # ---------------- scatter phase (both edge types)
                for t in (0, 1):
                    if l == 0:
                        tab_lo = tab_in[:]
                        tab_hi = tab_in[N - IDX_LIMIT:] if N > IDX_LIMIT else None
                    else:
                        tab_lo = ag_out[l - 1][:]
                        tab_hi = ag_out[l - 1][N - IDX_LIMIT:] \
                            if N > IDX_LIMIT else None

                    sch = schs[t]
                    # chunk meta: (region, window, win_first, win_last,
                    #              group_first, group_last); groups = 4 windows
                    # of one region sharing a [128,512] PSUM bank.
                    chunk_meta = []
                    for si, (r, w_, k) in enumerate(sch):
                        gf = (w_ % 4 == 0) or si == 0 or sch[si - 1][0] != r
                        gl = (w_ % 4 == 3) or si == len(sch) - 1 \
                            or sch[si + 1][0] != r
                        for j in range(k):
                            chunk_meta.append(
                                (r, w_, j == 0, j == k - 1,
                                 gf and j == 0, gl and j == k - 1))

                    pieces = _pieces(sch)
                    acc = None
                    for (p0, cnt, r) in pieces:
                        msg = msgp.tile([128, PIECE // 128, 128], bf16,
                                        tag="msg")
                        src_ap = tab_lo if r == 0 else tab_hi
                        nc.gpsimd.dma_gather(
                            msg[:, :cnt // 128, :], src_ap,
                            idx_t[t][:, p0 // 16:(p0 + cnt) // 16],
                            num_idxs=cnt, num_idxs_reg=cnt, elem_size=D,
                        )
                        for ci in range(cnt // 128):
                            gc = p0 // 128 + ci
                            (cr, w_, first, last, gfirst, glast) = chunk_meta[gc]
                            oh = ohp.tile([128, 128], bf16, tag="oh")
                            nc.vector.tensor_scalar(
                                out=oh[:], in0=iota_t[:],
                                scalar1=rel_t[t][:, gc:gc + 1],
                                scalar2=coef_t[t][:, gc:gc + 1],
                                op0=OP.is_equal, op1=OP.mult,
                            )
                            if gfirst:
                                acc = psA.tile([128, 512], f32, space="PSUM",
                                               tag="sc")
                            ws = (w_ % 4) * 128
                            nc.tensor.matmul(out=acc[:, ws:ws + 128],
                                             lhsT=msg[:, ci, :],
                                             rhs=oh[:], start=first, stop=last)
                            if glast:
                                base = (w_ // 4) * 512
                                wd = min(512, S - base)
                                # evacuate scatter PSUM on Act; each region
                                # gets its own fp16 accumulator and the dense
                                # phase sums them (extra matmul pair)
                                dst = gbf_t[t] if cr == 0 else grf_t[t]
                                nc.scalar.activation(
                                    out=dst[:, base:base + wd],
                                    in_=acc[:, :wd], func=AF.Identity)

                # ---------------- dense + stats partials
                sum_p = scp.tile([128, NF], f32, tag="sump")
                ssq_p = scp.tile([128, NF], f32, tag="ssqp")
                for ft in range(NF):
                    fw = min(512, S - ft * 512)
                    sl = slice(ft * 512, ft * 512 + fw)
                    # self-loop tiles: sl_t = dinv^2 * h  (fp16, 4x DVE)
                    slt = [scp.tile([128, 512], bf16, tag=f"sl{t}", bufs=3,
                                    name=f"sl{t}") for t in (0, 1)]
                    for t in (0, 1):
                        nc.vector.tensor_tensor(out=slt[t][:, :fw],
                                                in0=h16_t[:, sl],
                                                in1=dinvsq_t[t][:, sl],
                                                op=OP.mult)
                    dp = psB.tile([128, 512], f32, space="PSUM", tag="dense")
                    rhss = [(0, gbf_t[0][:, sl]), (1, gbf_t[1][:, sl]),
                            (0, slt[0][:, :fw]), (1, slt[1][:, :fw])]
                    if has_r1:
                        rhss += [(0, grf_t[0][:, sl]), (1, grf_t[1][:, sl])]
                    for mi, (t, rhs_ap) in enumerate(rhss):
                        nc.tensor.matmul(out=dp[:, :fw],
                                         lhsT=wd_t[:, l * 2 + t, :],
                                         rhs=rhs_ap, start=(mi == 0),
                                         stop=(mi == len(rhss) - 1))
                    nc.scalar.activation(out=h_pre[:, sl], in_=dp[:, :fw],
                                         func=AF.Identity,
                                         accum_out=sum_p[:, ft:ft + 1])
                    sq = scp.tile([128, 512], bf16, tag="sq", bufs=3)
                    nc.scalar.activation(out=sq[:, :fw], in_=dp[:, :fw],
                                         func=AF.Square,
                                         accum_out=ssq_p[:, ft:ft + 1])

                # ---------------- BN stats allreduce
                st = scp.tile([128, 2], f32, tag="st")
                nc.vector.tensor_reduce(out=st[:, 0:1], in_=sum_p[:],
                                        axis=mybir.AxisListType.X, op=OP.add)
                nc.vector.tensor_reduce(out=st[:, 1:2], in_=ssq_p[:],
                                        axis=mybir.AxisListType.X, op=OP.add)
                nc.sync.dma_start(st_in[l][:], st[:])
                nc.gpsimd.collective_compute(
                    "AllReduce", OP.add, replica_groups=rg,
                    ins=[st_in[l].opt()], outs=[st_out[l].opt()])
                sta = scp.tile([128, 2], f32, tag="sta")
                nc.sync.dma_start(sta[:], st_out[l][:])

                mean = scp.tile([128, 1], f32, tag="mean")
                var = scp.tile([128, 1], f32, tag="var")
                scl = scp.tile([128, 1], f32, tag="scl")
                sht = scp.tile([128, 1], f32, tag="sht")
                tmp = scp.tile([128, 1], f32, tag="tmp1")
                inv_n = 1.0 / float(N)
                nc.vector.tensor_scalar(out=mean[:], in0=sta[:, 0:1],
                                        scalar1=inv_n, scalar2=None, op0=OP.mult)
                nc.vector.tensor_scalar(out=var[:], in0=sta[:, 1:2],
                                        scalar1=inv_n, scalar2=None, op0=OP.mult)
                nc.vector.tensor_tensor(out=tmp[:], in0=mean[:], in1=mean[:],
                                        op=OP.mult)
                nc.vector.tensor_tensor(out=var[:], in0=var[:], in1=tmp[:],
                                        op=OP.subtract)
                # scl = gamma / sqrt(var + eps); sht = beta - mean*scl
                nc.vector.tensor_scalar(out=var[:], in0=var[:], scalar1=EPS_BN,
                                        scalar2=None, op0=OP.add)
                nc.scalar.activation(out=tmp[:], in_=var[:], func=AF.Sqrt)
                nc.vector.reciprocal(out=tmp[:], in_=tmp[:])
                nc.vector.tensor_tensor(out=scl[:], in0=gb_t[:, l:l + 1],
                                        in1=tmp[:], op=OP.mult)
                nc.vector.tensor_tensor(out=tmp[:], in0=mean[:], in1=scl[:],
                                        op=OP.mult)
                nc.vector.tensor_tensor(out=sht[:], in0=gb_t[:, L + l:L + l + 1],
                                        in1=tmp[:], op=OP.subtract)

                # ---------------- normalize (+ relu except last layer)
                if l < L - 1:
                    pass  # fused with the table build below (half-pipelined)
                else:
                    # tiled so the heads pipeline behind the normalize
                    for ft in range(NF):
                        fw = min(512, S - ft * 512)
                        sl = slice(ft * 512, ft * 512 + fw)
                        nc.scalar.activation(out=h16_t[:, sl],
                                             in_=h_pre[:, sl],
                                             func=AF.Identity,
                                             bias=sht[:], scale=scl[:])

                # ---------------- next-layer table: XBAR transpose + allgather
                # normalize/XBAR/table-DMA in halves so they pipeline
                # (disjoint column ranges -> no hazards between halves)
                if l < L - 1:
                    full = (S // 128) * 128
                    halves = [(0, 3072), (3072, SP_)] if S > 3072 \
                        else [(0, SP_)]
                    for (c0, c1) in halves:
                        ce = min(c1, S)
                        nc.scalar.activation(out=h16_t[:, c0:ce],
                                             in_=h_pre[:, c0:ce],
                                             func=AF.Relu,
                                             bias=sht[:], scale=scl[:])
                        nc.sync.dma_start_transpose(
                            stage[:, c0 // 128:c1 // 128, :],
                            h16_t[:, c0:c1])
                        de = min(c1, full)
                        nc.sync.dma_start(
                            ag_in[l][c0:de].rearrange("(w p) d -> p w d",
                                                      p=128),
                            stage[:, c0 // 128:de // 128, :])
                        if c1 > full and S > full:
                            nc.sync.dma_start(
                                ag_in[l][full:],
                                stage[:S - full, S // 128, :])
                    nc.gpsimd.collective_compute(
                        "AllGather", OP.bypass, replica_groups=rg,
                        ins=[ag_in[l].opt()], outs=[ag_out[l].opt()])

            # ---------------- heads (two activation-table passes)
            # pass 1: tanh embeddings, written straight to fp16 buffers
            for ft in range(NF):
                fw = min(512, S - ft * 512)
                sl = slice(ft * 512, ft * 512 + fw)
                e1p = psB.tile([128, 512], f32, space="PSUM", tag="dense")
                nc.tensor.matmul(out=e1p[:, :fw], lhsT=wh_t[:, 0, :],
                                 rhs=h16_t[:, sl], start=True, stop=True)
                nc.scalar.activation(out=e1b_t[:, sl], in_=e1p[:, :fw],
                                     func=AF.Tanh, bias=hb_t[:, 0:1])
                nc.sync.dma_start(outs[0][:, sl], e1b_t[:, sl])
                e2p = psB.tile([128, 512], f32, space="PSUM", tag="dense")
                nc.tensor.matmul(out=e2p[:, :fw], lhsT=wh_t[:, 1, :],
                                 rhs=h16_t[:, sl], start=True, stop=True)
                nc.scalar.activation(out=t2b_t[:, sl], in_=e2p[:, :fw],
                                     func=AF.Tanh, bias=hb_t[:, 1:2])

            # pass 2: l2norms + projection MLPs (sqrt activation set).
            # 1/max(||x||, eps) == 1/sqrt(||x||^2 + eps^2) via the Sqrt bias.
            def inv_norm(x_ap, fw):
                # all-fp16 chain: TensorTensor only has a 2x mode and only
                # for pure 2-byte operands, so keep everything fp16
                sq16 = scp.tile([128, 512], bf16, tag="sqb", bufs=4)
                nc.vector.tensor_tensor(out=sq16[:, :fw], in0=x_ap,
                                        in1=x_ap, op=OP.mult)
                nsq = psA.tile([128, 512], f32, space="PSUM", tag="sc")
                nc.tensor.matmul(out=nsq[:, :fw], lhsT=ones_t[:],
                                 rhs=sq16[:, :fw], start=True, stop=True)
                nrm = scp.tile([128, 512], bf16, tag="nrm", bufs=4)
                nc.scalar.activation(out=nrm[:, :fw], in_=nsq[:, :fw],
                                     func=AF.Sqrt, bias=epsn_t[:])
                with nc.allow_low_precision(reason="fp16 1/norm is plenty"):
                    nc.vector.reciprocal(out=nrm[:, :fw], in_=nrm[:, :fw])
                return nrm

            # pass 2a: e2 = l2norm(tanh) scaled in place in t2b
            for ft in range(NF):
                fw = min(512, S - ft * 512)
                sl = slice(ft * 512, ft * 512 + fw)
                nrm = inv_norm(t2b_t[:, sl], fw)
                nc.vector.tensor_tensor(out=t2b_t[:, sl], in0=t2b_t[:, sl],
                                        in1=nrm[:, :fw], op=OP.mult)
                nc.sync.dma_start(outs[1][:, sl], t2b_t[:, sl])

            # pass 2b: p1 projection (2 PSUM banks/tile -> 2-tile overlap)
            for ft in range(NF):
                fw = min(512, S - ft * 512)
                sl = slice(ft * 512, ft * 512 + fw)
                r1p = psB.tile([128, 512], f32, space="PSUM", tag="dense")
                nc.tensor.matmul(out=r1p[:, :fw], lhsT=wh_t[:, 2, :],
                                 rhs=e1b_t[:, sl], start=True, stop=True)
                r1b = scp.tile([128, 512], bf16, tag="r1b", bufs=3)
                nc.scalar.activation(out=r1b[:, :fw], in_=r1p[:, :fw],
                                     func=AF.Relu, bias=hb_t[:, 2:3])
                z1p = psB.tile([128, 512], f32, space="PSUM", tag="dense")
                nc.tensor.matmul(out=z1p[:, :fw], lhsT=wh_t[:, 3, :],
                                 rhs=r1b[:, :fw], start=True, stop=True)
                z1s = scp.tile([128, 512], bf16, tag="z1s", bufs=3)
                nc.vector.tensor_scalar(out=z1s[:, :fw], in0=z1p[:, :fw],
                                        scalar1=hb_t[:, 3:4], scalar2=None,
                                        op0=OP.add)
                nrm1 = inv_norm(z1s[:, :fw], fw)
                p1s = scp.tile([128, 512], bf16, tag="p1s", bufs=3)
                nc.vector.tensor_tensor(out=p1s[:, :fw], in0=z1s[:, :fw],
                                        in1=nrm1[:, :fw], op=OP.mult)
                nc.sync.dma_start(outs[2][:, sl], p1s[:, :fw])

            # pass 2c: p2 projection from the in-place e2 in t2b
            for ft in range(NF):
                fw = min(512, S - ft * 512)
                sl = slice(ft * 512, ft * 512 + fw)
                r2p = psB.tile([128, 512], f32, space="PSUM", tag="dense")
                nc.tensor.matmul(out=r2p[:, :fw], lhsT=wh_t[:, 4, :],
                                 rhs=t2b_t[:, sl], start=True, stop=True)
                r2b = scp.tile([128, 512], bf16, tag="r2b", bufs=3)
                nc.scalar.activation(out=r2b[:, :fw], in_=r2p[:, :fw],
                                     func=AF.Relu, bias=hb_t[:, 4:5])
                z2p = psB.tile([128, 512], f32, space="PSUM", tag="dense")
                nc.tensor.matmul(out=z2p[:, :fw], lhsT=wh_t[:, 5, :],
                                 rhs=r2b[:, :fw], start=True, stop=True)
                z2s = scp.tile([128, 512], bf16, tag="z2s", bufs=3)
                nc.vector.tensor_scalar(out=z2s[:, :fw], in0=z2p[:, :fw],
                                        scalar1=hb_t[:, 5:6], scalar2=None,
                                        op0=OP.add)
                nrm2 = inv_norm(z2s[:, :fw], fw)
                p2s = scp.tile([128, 512], bf16, tag="p2s", bufs=3)
                nc.vector.tensor_tensor(out=p2s[:, :fw], in0=z2s[:, :fw],
                                        in1=nrm2[:, :fw], op=OP.mult)
                nc.sync.dma_start(outs[3][:, sl], p2s[:, :fw])

    nc.compile()
    return nc


# ---------------------------------------------------------------- entry point

def _run(inputs, trace=False, trace_kwargs=None, nc_out=None):
    x = np.asarray(inputs["x"], np.float32)
    N = x.shape[0]
    assert N % NCORES == 0
    S = N // NCORES

    d0 = _prep_type(inputs["edge_index0"], N, S)
    d1 = _prep_type(inputs["edge_index1"], N, S)
    (dinv0, dinvsq0, sch0, idx0, rel0, coef0, stot0, nch0) = d0
    (dinv1, dinvsq1, sch1, idx1, rel1, coef1, stot1, nch1) = d1

    nc = _build(N, S, sch0, stot0, nch0, sch1, stot1, nch1)
    if nc_out is not None:
        nc_out.append(nc)

    tab = x.astype(np.float16)

    W0 = np.asarray(inputs["W0"], np.float32)
    W1 = np.asarray(inputs["W1"], np.float32)
    wd = np.zeros((L * 2 * 128, D), np.float32)
    for l in range(L):
        wd[(l * 2) * 128:(l * 2 + 1) * 128] = W0[l]
        wd[(l * 2 + 1) * 128:(l * 2 + 2) * 128] = W1[l]
    gb = np.stack([np.asarray(inputs["gamma"], np.float32).T,
                   np.asarray(inputs["beta"], np.float32).T], 0)
    gb = np.concatenate([gb[0], gb[1]], axis=1)  # [128, 2L]
    wh = np.concatenate([np.asarray(inputs[k], np.float32) for k in
                         ("emb1_W", "emb2_W", "ph1_Wa", "ph1_Wb",
                          "ph2_Wa", "ph2_Wb")], 0)
    hb = np.stack([np.asarray(inputs[k], np.float32) for k in
                   ("emb1_b", "emb2_b", "ph1_ba", "ph1_bb",
                    "ph2_ba", "ph2_bb")], 1)

    iota = np.broadcast_to(np.arange(128, dtype=np.float32),
                           (128, 128)).astype(np.float16)
    ones = np.ones((128, 128), np.float16)

    in_maps = []
    for c in range(NCORES):
        sl = slice(c * S, (c + 1) * S)
        in_maps.append({
            "tab_in": tab,
            "xT_in": np.ascontiguousarray(x[sl].T).astype(np.float16),
            "dinvsq0_in": np.ascontiguousarray(
                np.broadcast_to(dinvsq0[sl], (128, S))).astype(np.float16),
            "dinvsq1_in": np.ascontiguousarray(
                np.broadcast_to(dinvsq1[sl], (128, S))).astype(np.float16),
            "idx0_in": idx0[c], "idx1_in": idx1[c],
            "rel0_in": rel0[c], "rel1_in": rel1[c],
            "coef0_in": coef0[c], "coef1_in": coef1[c],
            "wd_in": wd.astype(np.float16),
            "gb_in": gb, "wh_in": wh.astype(np.float16), "hb_in": hb,
            "iota_in": iota, "ones_in": ones,
        })

    res = run_bass_kernel_spmd(nc, in_maps, list(range(NCORES)),
                               trace=trace, **(trace_kwargs or {}))

    full = {}
    for name in ("e1_o", "e2_o", "p1_o", "p2_o"):
        full[name] = np.concatenate(
            [res.results[c][name].T.astype(np.float32)
             for c in range(NCORES)], axis=0)
    return (full["e1_o"], full["e2_o"], full["p1_o"], full["p2_o"]), res


def kernel(**inputs):
    out, _ = _run(inputs)
    return out


# revision 71
# speedup vs baseline: 1.1393x; 1.0146x over previous
"""Trainium2 Bass kernel for nn_ClusterGCN (3-layer 2-edge-type GCN + heads).

Strategy (8 NeuronCores, node-parallel):
  - Nodes sharded contiguously: core c owns rows [c*S, (c+1)*S), S = N/8.
  - ONE replicated node-major fp16 table of h per layer (not per edge type);
    the full GCN edge coefficient dinv[src]*dinv[dst] is folded into the
    one-hot scatter matrix, so a single AllGather per layer boundary
    suffices. Edges are sharded by dst; messages h[src] are fetched with
    GPSIMD dma_gather (int16 indices, lo/hi base split for N > 32768) and
    scatter-added into a feature-major accumulator via one-hot matmuls on
    the PE: out[feat, dst_slot] += msg[edge, feat]^T @ (coef * onehot).
  - g_t = scatter_t + dinv_t^2 * h adds the self-loop, then
    h' = BN(g0 @ W0 + g1 @ W1) with batch stats AllReduced across cores.
  - Next-layer tables are built with a single XBAR dma transpose of the
    normalized h16 and AllGathered. Heads (tanh/relu/l2norm MLPs) run
    node-sharded in two activation-table passes (tanh pass, sqrt pass).
"""
import math
import numpy as np
import ml_dtypes

import concourse.bacc as bacc
import concourse.bass as bass
import concourse.mybir as mybir
import concourse.tile as tile
from concourse.library_config import mlp as mlp_lib
from concourse.bass_utils import run_bass_kernel_spmd

NCORES = 8
D = 128
L = 3
EPS_BN = 1e-5
EPS_NORM = 1e-12
IDX_LIMIT = 32768
PIECE = 1024          # gather slots per dma_gather instruction (ring sized to fit)
SENT_DST = 320.0      # sentinel dst slot (fp16-exact, >= 128)

f32 = mybir.dt.float32
bf16 = mybir.dt.float16  # (fp16 everywhere: 8x finer mantissa than bf16, same HW rates)
i16 = mybir.dt.int16
AF = mybir.ActivationFunctionType
OP = mybir.AluOpType


# ---------------------------------------------------------------- host prep

def _prep_type(edge_index, N, S):
    """Per edge type: degrees + per-core common-shape gather/scatter schedule."""
    src = np.asarray(edge_index[0], np.int64)
    dst = np.asarray(edge_index[1], np.int64)
    deg = np.bincount(dst, minlength=N).astype(np.float64) + 1.0
    dinv = 1.0 / np.sqrt(deg)

    NW = (S + 127) // 128
    HI_BASE = N - IDX_LIMIT  # hi-region table base; rows [HI_BASE, N)
    # src in [0, IDX_LIMIT) reachable from region 0; [HI_BASE, N) from region 1.
    # srcs in the overlap [max(HI_BASE,0), IDX_LIMIT) are flexible - used to
    # round region-0 groups up to full chunks and minimize sentinel padding.
    cores = []
    for c in range(NCORES):
        m = (dst >= c * S) & (dst < (c + 1) * S)
        s_c = src[m]
        d_c = dst[m]
        dl = d_c - c * S
        w = dl // 128
        order = np.lexsort((s_c, dl, w))
        cores.append((s_c[order], dl[order], w[order], d_c[order]))

    if N > IDX_LIMIT:
        K = np.zeros((2, NW), np.int64)
        must_lo = []
        for (s_c, dl, w, _) in cores:
            cnt_lo = np.bincount(w[s_c < HI_BASE], minlength=NW)
            must_lo.append(cnt_lo)
            K[0] = np.maximum(K[0], (cnt_lo + 127) // 128)
        K[0] = np.maximum(K[0], 1)
        core_reg = []
        for ci, (s_c, dl, w, _) in enumerate(cores):
            reg = (s_c >= IDX_LIMIT).astype(np.int64)
            for w_ in range(NW):
                cap = K[0][w_] * 128
                flex = np.flatnonzero((w == w_) & (s_c >= HI_BASE) & (s_c < IDX_LIMIT))
                take = min(max(cap - int(must_lo[ci][w_]), 0), len(flex))
                reg[flex[:take]] = 0
                reg[flex[take:]] = 1
            cnt_hi = np.bincount(w[reg == 1], minlength=NW)
            K[1] = np.maximum(K[1], (cnt_hi + 127) // 128)
            core_reg.append(reg)
        K[1] = np.maximum(K[1], 1)
        cores = [(s_c, dl, w, d_c, core_reg[ci])
                 for ci, (s_c, dl, w, d_c) in enumerate(cores)]
    else:
        K = np.zeros((2, NW), np.int64)
        for (s_c, dl, w, _) in cores:
            cnt = np.bincount(w, minlength=NW)
            K[0] = np.maximum(K[0], (cnt + 127) // 128)
        K[0] = np.maximum(K[0], 1)
        cores = [(s_c, dl, w, d_c, np.zeros(len(s_c), np.int64))
                 for (s_c, dl, w, d_c) in cores]

    schedule = []  # (region, window, nchunks) in slot order
    for r in (0, 1):
        for w_ in range(NW):
            if K[r][w_] > 0:
                schedule.append((r, int(w_), int(K[r][w_])))
    nchunks = sum(k for _, _, k in schedule)
    stot = nchunks * 128

    idx_all = np.zeros((NCORES, max(stot, 128)), np.int64)
    rel_all = np.full((NCORES, max(nchunks, 1) * 128), SENT_DST, np.float64)
    coef_all = np.zeros((NCORES, max(nchunks, 1) * 128), np.float64)
    for ci, (s_c, dl, w, d_c, reg) in enumerate(cores):
        pos = 0
        for (r, w_, k) in schedule:
            m = (reg == r) & (w == w_)
            n = int(m.sum())
            sv = s_c[m]
            idx_all[ci, pos:pos + n] = sv if r == 0 else sv - (N - IDX_LIMIT)
            rel_all[ci, pos:pos + n] = dl[m] - w_ * 128
            coef_all[ci, pos:pos + n] = dinv[sv] * dinv[d_c[m]]
            pos += k * 128

    # wrapped int16 index layout: idxs[p, s] = idx[s*16 + p%16]
    cols = max(stot // 16, 1)
    idx_w = np.zeros((NCORES, 128, cols), np.int16)
    if stot:
        a = idx_all[:, :stot].reshape(NCORES, cols, 16)  # [c, s, j]
        for p in range(128):
            idx_w[:, p, :] = a[:, :, p % 16]
    # dst-slot / coefficient tiles: [p, chunk] = value of edge chunk*128+p
    rel_t = np.ascontiguousarray(
        rel_all[:, :nchunks * 128].reshape(NCORES, nchunks, 128).transpose(0, 2, 1)
    ).astype(np.float32)
    coef_t = np.ascontiguousarray(
        coef_all[:, :nchunks * 128].reshape(NCORES, nchunks, 128).transpose(0, 2, 1)
    ).astype(np.float32)

    dinvsq = (dinv * dinv).astype(np.float32)
    return dinv.astype(np.float32), dinvsq, schedule, idx_w, rel_t, coef_t, stot, nchunks


def _pieces(schedule):
    """Split slot range into gather pieces that do not cross the lo/hi boundary.
    Returns list of (slot_start, slot_count, region)."""
    out = []
    for r in (0, 1):
        lo = sum(k * 128 for (rr, _, k) in schedule if rr < r)
        n = sum(k * 128 for (rr, _, k) in schedule if rr == r)
        p = lo
        while p < lo + n:
            c = min(PIECE, lo + n - p)
            out.append((p, c, r))
            p += c
    return out


# ---------------------------------------------------------------- device build

def _build(N, S, sch0, stot0, nch0, sch1, stot1, nch1):
    NW = (S + 127) // 128
    SP_ = NW * 128           # padded S (multiple of 128) for the XBAR transpose
    NF = (S + 511) // 512    # 512-wide node tiles
    nc = bacc.Bacc("TRN2", target_bir_lowering=False, debug=False,
                   num_devices=NCORES, dynamic_dma_scratch_size=PIECE * 16)

    def din(name, shape, dt):
        return nc.dram_tensor(name, shape, dt, kind="ExternalInput")

    tab_in = din("tab_in", [N, D], bf16)
    xT_in = din("xT_in", [128, S], bf16)
    dinvsq_in = [din("dinvsq0_in", [128, S], bf16), din("dinvsq1_in", [128, S], bf16)]
    idx_in = [din("idx0_in", [128, max(stot0 // 16, 1)], i16),
              din("idx1_in", [128, max(stot1 // 16, 1)], i16)]
    rel_in = [din("rel0_in", [128, max(nch0, 1)], f32),
              din("rel1_in", [128, max(nch1, 1)], f32)]
    coef_in = [din("coef0_in", [128, max(nch0, 1)], f32),
               din("coef1_in", [128, max(nch1, 1)], f32)]
    wd_in = din("wd_in", [L * 2 * 128, D], bf16)
    gb_in = din("gb_in", [128, 2 * L], f32)
    wh_in = din("wh_in", [6 * 128, D], bf16)
    hb_in = din("hb_in", [128, 6], f32)
    iota_in = din("iota_in", [128, 128], bf16)
    ones_in = din("ones_in", [128, 128], bf16)

    outs = [nc.dram_tensor(n, [128, S], bf16, kind="ExternalOutput")
            for n in ("e1_o", "e2_o", "p1_o", "p2_o")]

    with tile.TileContext(nc) as tc:
        with (
            tc.tile_pool(name="const", bufs=1) as const,
            tc.tile_pool(name="g", bufs=1) as gpool,
            tc.tile_pool(name="msg", bufs=8) as msgp,
            tc.tile_pool(name="oh", bufs=64) as ohp,
            tc.tile_pool(name="scr", bufs=1) as scp,
            tc.tile_pool(name="psA", bufs=4, space="PSUM") as psA,
            tc.tile_pool(name="psB", bufs=4, space="PSUM") as psB,
            tc.tile_pool(name="dram", bufs=1, space="DRAM") as dram,
        ):
            nc.gpsimd.load_library(mlp_lib)

            # ---- persistent SBUF tiles
            iota_t = const.tile([128, 128], bf16)
            ones_t = const.tile([128, 128], bf16)
            dinvsq_t = [const.tile([128, S], bf16, tag=f"dq{t}", name=f"dq{t}")
                        for t in (0, 1)]
            idx_t = [const.tile([128, max(stot0 // 16, 1)], i16, tag="idx0", name="idx0"),
                     const.tile([128, max(stot1 // 16, 1)], i16, tag="idx1", name="idx1")]
            rel_t = [const.tile([128, max(nch0, 1)], f32, tag="rel0", name="rel0"),
                     const.tile([128, max(nch1, 1)], f32, tag="rel1", name="rel1")]
            coef_t = [const.tile([128, max(nch0, 1)], f32, tag="coef0", name="coef0"),
                      const.tile([128, max(nch1, 1)], f32, tag="coef1", name="coef1")]
            wd_t = const.tile([128, L * 2, D], bf16)     # dense weights
            wh_t = const.tile([128, 6, D], bf16)         # head weights
            whp_t = const.tile([128, 2, D], bf16)        # BN-folded emb weights
            hbf_t = const.tile([128, 2], f32)            # BN-folded emb biases
            gb_t = const.tile([128, 2 * L], f32)
            hb_t = const.tile([128, 6], f32)
            epsn_t = const.tile([128, 1], f32)
            nc.vector.memset(epsn_t[:], EPS_NORM * EPS_NORM)

            h16_t = gpool.tile([128, SP_], bf16, tag="h16")      # padded cols
            h_pre = gpool.tile([128, S], f32, tag="hpre")
            gbf_t = [gpool.tile([128, S], bf16, tag=f"gbf{t}", name=f"gbf{t}")
                     for t in (0, 1)]
            # stage (layer boundaries) and e1b (heads) never live at the same
            # time; alias them in one padded buffer to save SBUF.
            ub = gpool.tile([128, SP_], bf16, tag="stage")
            stage = ub[:].rearrange("p (w k) -> p w k", k=128)
            e1b_t = ub[:]
            t2b_t = gpool.tile([128, S], bf16, tag="t2b")
            # region-1 scatter accumulators, aliased over boundary/heads-only
            # buffers (stage/ub and t2b are idle during scatter+dense)
            grf_t = [ub[:, :S], t2b_t[:]]

            nc.sync.dma_start(iota_t[:], iota_in[:])
            nc.sync.dma_start(ones_t[:], ones_in[:])
            for t in (0, 1):
                nc.sync.dma_start(dinvsq_t[t][:], dinvsq_in[t][:])
                nc.sync.dma_start(idx_t[t][:], idx_in[t][:])
                nc.sync.dma_start(rel_t[t][:], rel_in[t][:])
                nc.sync.dma_start(coef_t[t][:], coef_in[t][:])
            nc.sync.dma_start(
                wd_t[:], wd_in[:].rearrange("(k p) d -> p k d", p=128))
            nc.sync.dma_start(
                wh_t[:], wh_in[:].rearrange("(k p) d -> p k d", p=128))
            nc.sync.dma_start(gb_t[:], gb_in[:])
            nc.sync.dma_start(hb_t[:], hb_in[:])
            nc.sync.dma_start(h16_t[:, :S], xT_in[:])
            if SP_ > S:
                nc.vector.memset(h16_t[:, S:], 0.0)

            # ---- internal DRAM for collectives
            ag_in = {}
            ag_out = {}
            for l in (0, 1):
                ag_in[l] = dram.tile([S, D], bf16, tag=f"agi{l}", name=f"agi{l}")
                ag_out[l] = dram.tile([N, D], bf16, addr_space="Shared",
                                      tag=f"ago{l}", name=f"ago{l}")
            st_in = [dram.tile([128, 2], f32, tag=f"sti{l}", name=f"sti{l}")
                     for l in range(L)]
            st_out = [dram.tile([128, 2], f32, addr_space="Shared",
                                tag=f"sto{l}", name=f"sto{l}") for l in range(L)]

            schs = (sch0, sch1)
            has_r1 = any(r == 1 for (r, _, _) in sch0)
            rg = [list(range(NCORES))]

            for l in range(L):
                # ---------------- scatter phase (both edge types)
                for t in (0, 1):
                    if l == 0:
                        tab_lo = tab_in[:]
                        tab_hi = tab_in[N - IDX_LIMIT:] if N > IDX_LIMIT else None
                    else:
                        tab_lo = ag_out[l - 1][:]
                        tab_hi = ag_out[l - 1][N - IDX_LIMIT:] \
                            if N > IDX_LIMIT else None

                    sch = schs[t]
                    # chunk meta: (region, window, win_first, win_last,
                    #              group_first, group_last); groups = 4 windows
                    # of one region sharing a [128,512] PSUM bank.
                    chunk_meta = []
                    for si, (r, w_, k) in enumerate(sch):
                        gf = (w_ % 4 == 0) or si == 0 or sch[si - 1][0] != r
                        gl = (w_ % 4 == 3) or si == len(sch) - 1 \
                            or sch[si + 1][0] != r
                        for j in range(k):
                            chunk_meta.append(
                                (r, w_, j == 0, j == k - 1,
                                 gf and j == 0, gl and j == k - 1))

                    pieces = _pieces(sch)
                    acc = None
                    for (p0, cnt, r) in pieces:
                        msg = msgp.tile([128, PIECE // 128, 128], bf16,
                                        tag="msg")
                        src_ap = tab_lo if r == 0 else tab_hi
                        nc.gpsimd.dma_gather(
                            msg[:, :cnt // 128, :], src_ap,
                            idx_t[t][:, p0 // 16:(p0 + cnt) // 16],
                            num_idxs=cnt, num_idxs_reg=cnt, elem_size=D,
                        )
                        for ci in range(cnt // 128):
                            gc = p0 // 128 + ci
                            (cr, w_, first, last, gfirst, glast) = chunk_meta[gc]
                            oh = ohp.tile([128, 128], bf16, tag="oh")
                            nc.vector.tensor_scalar(
                                out=oh[:], in0=iota_t[:],
                                scalar1=rel_t[t][:, gc:gc + 1],
                                scalar2=coef_t[t][:, gc:gc + 1],
                                op0=OP.is_equal, op1=OP.mult,
                            )
                            if gfirst:
                                acc = psA.tile([128, 512], f32, space="PSUM",
                                               tag="sc")
                            ws = (w_ % 4) * 128
                            nc.tensor.matmul(out=acc[:, ws:ws + 128],
                                             lhsT=msg[:, ci, :],
                                             rhs=oh[:], start=first, stop=last)
                            if glast:
                                base = (w_ // 4) * 512
                                wd = min(512, S - base)
                                # evacuate scatter PSUM on Act; each region
                                # gets its own fp16 accumulator and the dense
                                # phase sums them (extra matmul pair)
                                dst = gbf_t[t] if cr == 0 else grf_t[t]
                                nc.scalar.activation(
                                    out=dst[:, base:base + wd],
                                    in_=acc[:, :wd], func=AF.Identity)

                # ---------------- dense + stats partials
                sum_p = scp.tile([128, NF], f32, tag="sump")
                ssq_p = scp.tile([128, NF], f32, tag="ssqp")
                for ft in range(NF):
                    fw = min(512, S - ft * 512)
                    sl = slice(ft * 512, ft * 512 + fw)
                    # self-loop tiles: sl_t = dinv^2 * h  (fp16, 4x DVE)
                    slt = [scp.tile([128, 512], bf16, tag=f"sl{t}", bufs=3,
                                    name=f"sl{t}") for t in (0, 1)]
                    for t in (0, 1):
                        nc.vector.tensor_tensor(out=slt[t][:, :fw],
                                                in0=h16_t[:, sl],
                                                in1=dinvsq_t[t][:, sl],
                                                op=OP.mult)
                    dp = psB.tile([128, 512], f32, space="PSUM", tag="dense")
                    rhss = [(0, gbf_t[0][:, sl]), (1, gbf_t[1][:, sl]),
                            (0, slt[0][:, :fw]), (1, slt[1][:, :fw])]
                    if has_r1:
                        rhss += [(0, grf_t[0][:, sl]), (1, grf_t[1][:, sl])]
                    for mi, (t, rhs_ap) in enumerate(rhss):
                        nc.tensor.matmul(out=dp[:, :fw],
                                         lhsT=wd_t[:, l * 2 + t, :],
                                         rhs=rhs_ap, start=(mi == 0),
                                         stop=(mi == len(rhss) - 1))
                    nc.scalar.activation(out=h_pre[:, sl], in_=dp[:, :fw],
                                         func=AF.Identity,
                                         accum_out=sum_p[:, ft:ft + 1])
                    sq = scp.tile([128, 512], bf16, tag="sq", bufs=3)
                    nc.scalar.activation(out=sq[:, :fw], in_=dp[:, :fw],
                                         func=AF.Square,
                                         accum_out=ssq_p[:, ft:ft + 1])

                # ---------------- BN stats allreduce
                st = scp.tile([128, 2], f32, tag="st")
                nc.vector.tensor_reduce(out=st[:, 0:1], in_=sum_p[:],
                                        axis=mybir.AxisListType.X, op=OP.add)
                nc.vector.tensor_reduce(out=st[:, 1:2], in_=ssq_p[:],
                                        axis=mybir.AxisListType.X, op=OP.add)
                nc.sync.dma_start(st_in[l][:], st[:])
                nc.gpsimd.collective_compute(
                    "AllReduce", OP.add, replica_groups=rg,
                    ins=[st_in[l].opt()], outs=[st_out[l].opt()])
                if l == L - 1:
                    # raw fp16 copy of the dense output, hidden in the
                    # AllReduce window (the BN affine folds into the heads)
                    nc.scalar.activation(out=h16_t[:, :S], in_=h_pre[:],
                                         func=AF.Identity)
                sta = scp.tile([128, 2], f32, tag="sta")
                nc.sync.dma_start(sta[:], st_out[l][:])

                mean = scp.tile([128, 1], f32, tag="mean")
                var = scp.tile([128, 1], f32, tag="var")
                scl = scp.tile([128, 1], f32, tag="scl")
                sht = scp.tile([128, 1], f32, tag="sht")
                tmp = scp.tile([128, 1], f32, tag="tmp1")
                inv_n = 1.0 / float(N)
                nc.vector.tensor_scalar(out=mean[:], in0=sta[:, 0:1],
                                        scalar1=inv_n, scalar2=None, op0=OP.mult)
                nc.vector.tensor_scalar(out=var[:], in0=sta[:, 1:2],
                                        scalar1=inv_n, scalar2=None, op0=OP.mult)
                nc.vector.tensor_tensor(out=tmp[:], in0=mean[:], in1=mean[:],
                                        op=OP.mult)
                nc.vector.tensor_tensor(out=var[:], in0=var[:], in1=tmp[:],
                                        op=OP.subtract)
                # scl = gamma / sqrt(var + eps); sht = beta - mean*scl
                nc.vector.tensor_scalar(out=var[:], in0=var[:], scalar1=EPS_BN,
                                        scalar2=None, op0=OP.add)
                nc.scalar.activation(out=tmp[:], in_=var[:], func=AF.Sqrt)
                nc.vector.reciprocal(out=tmp[:], in_=tmp[:])
                nc.vector.tensor_tensor(out=scl[:], in0=gb_t[:, l:l + 1],
                                        in1=tmp[:], op=OP.mult)
                nc.vector.tensor_tensor(out=tmp[:], in0=mean[:], in1=scl[:],
                                        op=OP.mult)
                nc.vector.tensor_tensor(out=sht[:], in0=gb_t[:, L + l:L + l + 1],
                                        in1=tmp[:], op=OP.subtract)

                # ---------------- normalize (+ relu except last layer)
                if l < L - 1:
                    pass  # fused with the table build below (half-pipelined)
                else:
                    # last layer has no relu: fold the BN affine into the
                    # head weights instead of normalizing h. h16 := fp16(raw
                    # h_pre) was already copied during the AllReduce window;
                    # whp_t = wh*scl and the tanh biases absorb wh^T @ sht.
                    sht16 = scp.tile([128, 1], bf16, tag="sht16")
                    nc.vector.tensor_copy(out=sht16[:], in_=sht[:])
                    for k in (0, 1):
                        nc.scalar.activation(out=whp_t[:, k, :],
                                             in_=wh_t[:, k, :],
                                             func=AF.Identity, scale=scl[:])
                        bp = psA.tile([128, 512], f32, space="PSUM", tag="sc")
                        nc.tensor.matmul(out=bp[:, 0:1], lhsT=wh_t[:, k, :],
                                         rhs=sht16[:], start=True, stop=True)
                        nc.vector.tensor_tensor(out=hbf_t[:, k:k + 1],
                                                in0=hb_t[:, k:k + 1],
                                                in1=bp[:, 0:1], op=OP.add)

                # ---------------- next-layer table: XBAR transpose + allgather
                # normalize/XBAR/table-DMA in halves so they pipeline
                # (disjoint column ranges -> no hazards between halves)
                if l < L - 1:
                    full = (S // 128) * 128
                    halves = [(0, 3072), (3072, SP_)] if S > 3072 \
                        else [(0, SP_)]
                    for (c0, c1) in halves:
                        ce = min(c1, S)
                        nc.scalar.activation(out=h16_t[:, c0:ce],
                                             in_=h_pre[:, c0:ce],
                                             func=AF.Relu,
                                             bias=sht[:], scale=scl[:])
                        nc.sync.dma_start_transpose(
                            stage[:, c0 // 128:c1 // 128, :],
                            h16_t[:, c0:c1])
                        de = min(c1, full)
                        nc.sync.dma_start(
                            ag_in[l][c0:de].rearrange("(w p) d -> p w d",
                                                      p=128),
                            stage[:, c0 // 128:de // 128, :])
                        if c1 > full and S > full:
                            nc.sync.dma_start(
                                ag_in[l][full:],
                                stage[:S - full, S // 128, :])
                    nc.gpsimd.collective_compute(
                        "AllGather", OP.bypass, replica_groups=rg,
                        ins=[ag_in[l].opt()], outs=[ag_out[l].opt()])

            # ---------------- heads (two activation-table passes)
            # pass 1: tanh embeddings, written straight to fp16 buffers
            for ft in range(NF):
                fw = min(512, S - ft * 512)
                sl = slice(ft * 512, ft * 512 + fw)
                e1p = psB.tile([128, 512], f32, space="PSUM", tag="dense")
                nc.tensor.matmul(out=e1p[:, :fw], lhsT=whp_t[:, 0, :],
                                 rhs=h16_t[:, sl], start=True, stop=True)
                nc.scalar.activation(out=e1b_t[:, sl], in_=e1p[:, :fw],
                                     func=AF.Tanh, bias=hbf_t[:, 0:1])
                nc.sync.dma_start(outs[0][:, sl], e1b_t[:, sl])
                e2p = psB.tile([128, 512], f32, space="PSUM", tag="dense")
                nc.tensor.matmul(out=e2p[:, :fw], lhsT=whp_t[:, 1, :],
                                 rhs=h16_t[:, sl], start=True, stop=True)
                nc.scalar.activation(out=t2b_t[:, sl], in_=e2p[:, :fw],
                                     func=AF.Tanh, bias=hbf_t[:, 1:2])

            # pass 2: l2norms + projection MLPs (sqrt activation set).
            # 1/max(||x||, eps) == 1/sqrt(||x||^2 + eps^2) via the Sqrt bias.
            def inv_norm(x_ap, fw):
                # all-fp16 chain: TensorTensor only has a 2x mode and only
                # for pure 2-byte operands, so keep everything fp16
                sq16 = scp.tile([128, 512], bf16, tag="sqb", bufs=4)
                nc.vector.tensor_tensor(out=sq16[:, :fw], in0=x_ap,
                                        in1=x_ap, op=OP.mult)
                nsq = psA.tile([128, 512], f32, space="PSUM", tag="sc")
                nc.tensor.matmul(out=nsq[:, :fw], lhsT=ones_t[:],
                                 rhs=sq16[:, :fw], start=True, stop=True)
                nrm = scp.tile([128, 512], bf16, tag="nrm", bufs=4)
                nc.scalar.activation(out=nrm[:, :fw], in_=nsq[:, :fw],
                                     func=AF.Sqrt, bias=epsn_t[:])
                with nc.allow_low_precision(reason="fp16 1/norm is plenty"):
                    nc.vector.reciprocal(out=nrm[:, :fw], in_=nrm[:, :fw])
                return nrm

            # pass 2a: e2 = l2norm(tanh) scaled in place in t2b
            for ft in range(NF):
                fw = min(512, S - ft * 512)
                sl = slice(ft * 512, ft * 512 + fw)
                nrm = inv_norm(t2b_t[:, sl], fw)
                nc.vector.tensor_tensor(out=t2b_t[:, sl], in0=t2b_t[:, sl],
                                        in1=nrm[:, :fw], op=OP.mult)
                nc.sync.dma_start(outs[1][:, sl], t2b_t[:, sl])

            # pass 2b: p1 projection (2 PSUM banks/tile -> 2-tile overlap)
            for ft in range(NF):
                fw = min(512, S - ft * 512)
                sl = slice(ft * 512, ft * 512 + fw)
                r1p = psB.tile([128, 512], f32, space="PSUM", tag="dense")
                nc.tensor.matmul(out=r1p[:, :fw], lhsT=wh_t[:, 2, :],
                                 rhs=e1b_t[:, sl], start=True, stop=True)
                r1b = scp.tile([128, 512], bf16, tag="r1b", bufs=3)
                nc.scalar.activation(out=r1b[:, :fw], in_=r1p[:, :fw],
                                     func=AF.Relu, bias=hb_t[:, 2:3])
                z1p = psB.tile([128, 512], f32, space="PSUM", tag="dense")
                nc.tensor.matmul(out=z1p[:, :fw], lhsT=wh_t[:, 3, :],
                                 rhs=r1b[:, :fw], start=True, stop=True)
                z1s = scp.tile([128, 512], bf16, tag="z1s", bufs=3)
                nc.vector.tensor_scalar(out=z1s[:, :fw], in0=z1p[:, :fw],
                                        scalar1=hb_t[:, 3:4], scalar2=None,
                                        op0=OP.add)
                nrm1 = inv_norm(z1s[:, :fw], fw)
                p1s = scp.tile([128, 512], bf16, tag="p1s", bufs=3)
                nc.vector.tensor_tensor(out=p1s[:, :fw], in0=z1s[:, :fw],
                                        in1=nrm1[:, :fw], op=OP.mult)
                nc.sync.dma_start(outs[2][:, sl], p1s[:, :fw])

            # pass 2c: p2 projection from the in-place e2 in t2b
            for ft in range(NF):
                fw = min(512, S - ft * 512)
                sl = slice(ft * 512, ft * 512 + fw)
                r2p = psB.tile([128, 512], f32, space="PSUM", tag="dense")
                nc.tensor.matmul(out=r2p[:, :fw], lhsT=wh_t[:, 4, :],
                                 rhs=t2b_t[:, sl], start=True, stop=True)
                r2b = scp.tile([128, 512], bf16, tag="r2b", bufs=3)
                nc.scalar.activation(out=r2b[:, :fw], in_=r2p[:, :fw],
                                     func=AF.Relu, bias=hb_t[:, 4:5])
                z2p = psB.tile([128, 512], f32, space="PSUM", tag="dense")
                nc.tensor.matmul(out=z2p[:, :fw], lhsT=wh_t[:, 5, :],
                                 rhs=r2b[:, :fw], start=True, stop=True)
                z2s = scp.tile([128, 512], bf16, tag="z2s", bufs=3)
                nc.vector.tensor_scalar(out=z2s[:, :fw], in0=z2p[:, :fw],
                                        scalar1=hb_t[:, 5:6], scalar2=None,
                                        op0=OP.add)
                nrm2 = inv_norm(z2s[:, :fw], fw)
                p2s = scp.tile([128, 512], bf16, tag="p2s", bufs=3)
                nc.vector.tensor_tensor(out=p2s[:, :fw], in0=z2s[:, :fw],
                                        in1=nrm2[:, :fw], op=OP.mult)
                nc.sync.dma_start(outs[3][:, sl], p2s[:, :fw])

    nc.compile()
    return nc


# ---------------------------------------------------------------- entry point

def _run(inputs, trace=False, trace_kwargs=None, nc_out=None):
    x = np.asarray(inputs["x"], np.float32)
    N = x.shape[0]
    assert N % NCORES == 0
    S = N // NCORES

    d0 = _prep_type(inputs["edge_index0"], N, S)
    d1 = _prep_type(inputs["edge_index1"], N, S)
    (dinv0, dinvsq0, sch0, idx0, rel0, coef0, stot0, nch0) = d0
    (dinv1, dinvsq1, sch1, idx1, rel1, coef1, stot1, nch1) = d1

    nc = _build(N, S, sch0, stot0, nch0, sch1, stot1, nch1)
    if nc_out is not None:
        nc_out.append(nc)

    tab = x.astype(np.float16)

    W0 = np.asarray(inputs["W0"], np.float32)
    W1 = np.asarray(inputs["W1"], np.float32)
    wd = np.zeros((L * 2 * 128, D), np.float32)
    for l in range(L):
        wd[(l * 2) * 128:(l * 2 + 1) * 128] = W0[l]
        wd[(l * 2 + 1) * 128:(l * 2 + 2) * 128] = W1[l]
    gb = np.stack([np.asarray(inputs["gamma"], np.float32).T,
                   np.asarray(inputs["beta"], np.float32).T], 0)
    gb = np.concatenate([gb[0], gb[1]], axis=1)  # [128, 2L]
    wh = np.concatenate([np.asarray(inputs[k], np.float32) for k in
                         ("emb1_W", "emb2_W", "ph1_Wa", "ph1_Wb",
                          "ph2_Wa", "ph2_Wb")], 0)
    hb = np.stack([np.asarray(inputs[k], np.float32) for k in
                   ("emb1_b", "emb2_b", "ph1_ba", "ph1_bb",
                    "ph2_ba", "ph2_bb")], 1)

    iota = np.broadcast_to(np.arange(128, dtype=np.float32),
                           (128, 128)).astype(np.float16)
    ones = np.ones((128, 128), np.float16)

    in_maps = []
    for c in range(NCORES):
        sl = slice(c * S, (c + 1) * S)
        in_maps.append({
            "tab_in": tab,
            "xT_in": np.ascontiguousarray(x[sl].T).astype(np.float16),
            "dinvsq0_in": np.ascontiguousarray(
                np.broadcast_to(dinvsq0[sl], (128, S))).astype(np.float16),
            "dinvsq1_in": np.ascontiguousarray(
                np.broadcast_to(dinvsq1[sl], (128, S))).astype(np.float16),
            "idx0_in": idx0[c], "idx1_in": idx1[c],
            "rel0_in": rel0[c], "rel1_in": rel1[c],
            "coef0_in": coef0[c], "coef1_in": coef1[c],
            "wd_in": wd.astype(np.float16),
            "gb_in": gb, "wh_in": wh.astype(np.float16), "hb_in": hb,
            "iota_in": iota, "ones_in": ones,
        })

    res = run_bass_kernel_spmd(nc, in_maps, list(range(NCORES)),
                               trace=trace, **(trace_kwargs or {}))

    full = {}
    for name in ("e1_o", "e2_o", "p1_o", "p2_o"):
        full[name] = np.concatenate(
            [res.results[c][name].T.astype(np.float32)
             for c in range(NCORES)], axis=0)
    return (full["e1_o"], full["e2_o"], full["p1_o"], full["p2_o"]), res


def kernel(**inputs):
    out, _ = _run(inputs)
    return out


# revision 72
# speedup vs baseline: 1.1520x; 1.0111x over previous
"""Trainium2 Bass kernel for nn_ClusterGCN (3-layer 2-edge-type GCN + heads).

Strategy (8 NeuronCores, node-parallel):
  - Nodes sharded contiguously: core c owns rows [c*S, (c+1)*S), S = N/8.
  - ONE replicated node-major fp16 table of h per layer (not per edge type);
    the full GCN edge coefficient dinv[src]*dinv[dst] is folded into the
    one-hot scatter matrix, so a single AllGather per layer boundary
    suffices. Edges are sharded by dst; messages h[src] are fetched with
    GPSIMD dma_gather (int16 indices, lo/hi base split for N > 32768) and
    scatter-added into a feature-major accumulator via one-hot matmuls on
    the PE: out[feat, dst_slot] += msg[edge, feat]^T @ (coef * onehot).
  - g_t = scatter_t + dinv_t^2 * h adds the self-loop, then
    h' = BN(g0 @ W0 + g1 @ W1) with batch stats AllReduced across cores.
  - Next-layer tables are built with a single XBAR dma transpose of the
    normalized h16 and AllGathered. Heads (tanh/relu/l2norm MLPs) run
    node-sharded in two activation-table passes (tanh pass, sqrt pass).
"""
import math
import numpy as np
import ml_dtypes

import concourse.bacc as bacc
import concourse.bass as bass
import concourse.mybir as mybir
import concourse.tile as tile
from concourse.library_config import mlp as mlp_lib
from concourse.bass_utils import run_bass_kernel_spmd

NCORES = 8
D = 128
L = 3
EPS_BN = 1e-5
EPS_NORM = 1e-12
IDX_LIMIT = 32768
PIECE = 1024          # gather slots per dma_gather instruction (ring sized to fit)
SENT_DST = 320.0      # sentinel dst slot (fp16-exact, >= 128)

f32 = mybir.dt.float32
bf16 = mybir.dt.float16  # (fp16 everywhere: 8x finer mantissa than bf16, same HW rates)
i16 = mybir.dt.int16
AF = mybir.ActivationFunctionType
OP = mybir.AluOpType


# ---------------------------------------------------------------- host prep

def _prep_type(edge_index, N, S):
    """Per edge type: degrees + per-core common-shape gather/scatter schedule."""
    src = np.asarray(edge_index[0], np.int64)
    dst = np.asarray(edge_index[1], np.int64)
    deg = np.bincount(dst, minlength=N).astype(np.float64) + 1.0
    dinv = 1.0 / np.sqrt(deg)

    NW = (S + 127) // 128
    HI_BASE = N - IDX_LIMIT  # hi-region table base; rows [HI_BASE, N)
    # src in [0, IDX_LIMIT) reachable from region 0; [HI_BASE, N) from region 1.
    # srcs in the overlap [max(HI_BASE,0), IDX_LIMIT) are flexible - used to
    # round region-0 groups up to full chunks and minimize sentinel padding.
    cores = []
    for c in range(NCORES):
        m = (dst >= c * S) & (dst < (c + 1) * S)
        s_c = src[m]
        d_c = dst[m]
        dl = d_c - c * S
        w = dl // 128
        order = np.lexsort((s_c, dl, w))
        cores.append((s_c[order], dl[order], w[order], d_c[order]))

    if N > IDX_LIMIT:
        K = np.zeros((2, NW), np.int64)
        must_lo = []
        for (s_c, dl, w, _) in cores:
            cnt_lo = np.bincount(w[s_c < HI_BASE], minlength=NW)
            must_lo.append(cnt_lo)
            K[0] = np.maximum(K[0], (cnt_lo + 127) // 128)
        K[0] = np.maximum(K[0], 1)
        core_reg = []
        for ci, (s_c, dl, w, _) in enumerate(cores):
            reg = (s_c >= IDX_LIMIT).astype(np.int64)
            for w_ in range(NW):
                cap = K[0][w_] * 128
                flex = np.flatnonzero((w == w_) & (s_c >= HI_BASE) & (s_c < IDX_LIMIT))
                take = min(max(cap - int(must_lo[ci][w_]), 0), len(flex))
                reg[flex[:take]] = 0
                reg[flex[take:]] = 1
            cnt_hi = np.bincount(w[reg == 1], minlength=NW)
            K[1] = np.maximum(K[1], (cnt_hi + 127) // 128)
            core_reg.append(reg)
        K[1] = np.maximum(K[1], 1)
        cores = [(s_c, dl, w, d_c, core_reg[ci])
                 for ci, (s_c, dl, w, d_c) in enumerate(cores)]
    else:
        K = np.zeros((2, NW), np.int64)
        for (s_c, dl, w, _) in cores:
            cnt = np.bincount(w, minlength=NW)
            K[0] = np.maximum(K[0], (cnt + 127) // 128)
        K[0] = np.maximum(K[0], 1)
        cores = [(s_c, dl, w, d_c, np.zeros(len(s_c), np.int64))
                 for (s_c, dl, w, d_c) in cores]

    schedule = []  # (region, window, nchunks) in slot order
    for r in (0, 1):
        for w_ in range(NW):
            if K[r][w_] > 0:
                schedule.append((r, int(w_), int(K[r][w_])))
    nchunks = sum(k for _, _, k in schedule)
    stot = nchunks * 128

    idx_all = np.zeros((NCORES, max(stot, 128)), np.int64)
    rel_all = np.full((NCORES, max(nchunks, 1) * 128), SENT_DST, np.float64)
    coef_all = np.zeros((NCORES, max(nchunks, 1) * 128), np.float64)
    for ci, (s_c, dl, w, d_c, reg) in enumerate(cores):
        pos = 0
        for (r, w_, k) in schedule:
            m = (reg == r) & (w == w_)
            n = int(m.sum())
            sv = s_c[m]
            idx_all[ci, pos:pos + n] = sv if r == 0 else sv - (N - IDX_LIMIT)
            rel_all[ci, pos:pos + n] = dl[m] - w_ * 128
            coef_all[ci, pos:pos + n] = dinv[sv] * dinv[d_c[m]]
            pos += k * 128

    # wrapped int16 index layout: idxs[p, s] = idx[s*16 + p%16]
    cols = max(stot // 16, 1)
    idx_w = np.zeros((NCORES, 128, cols), np.int16)
    if stot:
        a = idx_all[:, :stot].reshape(NCORES, cols, 16)  # [c, s, j]
        for p in range(128):
            idx_w[:, p, :] = a[:, :, p % 16]
    # dst-slot / coefficient tiles: [p, chunk] = value of edge chunk*128+p
    rel_t = np.ascontiguousarray(
        rel_all[:, :nchunks * 128].reshape(NCORES, nchunks, 128).transpose(0, 2, 1)
    ).astype(np.float32)
    coef_t = np.ascontiguousarray(
        coef_all[:, :nchunks * 128].reshape(NCORES, nchunks, 128).transpose(0, 2, 1)
    ).astype(np.float32)

    dinvsq = (dinv * dinv).astype(np.float32)
    return dinv.astype(np.float32), dinvsq, schedule, idx_w, rel_t, coef_t, stot, nchunks


def _pieces(schedule):
    """Split slot range into gather pieces that do not cross the lo/hi boundary.
    Returns list of (slot_start, slot_count, region)."""
    out = []
    for r in (0, 1):
        lo = sum(k * 128 for (rr, _, k) in schedule if rr < r)
        n = sum(k * 128 for (rr, _, k) in schedule if rr == r)
        p = lo
        while p < lo + n:
            c = min(PIECE, lo + n - p)
            out.append((p, c, r))
            p += c
    return out


# ---------------------------------------------------------------- device build

def _build(N, S, sch0, stot0, nch0, sch1, stot1, nch1):
    NW = (S + 127) // 128
    SP_ = NW * 128           # padded S (multiple of 128) for the XBAR transpose
    NF = (S + 511) // 512    # 512-wide node tiles
    nc = bacc.Bacc("TRN2", target_bir_lowering=False, debug=False,
                   num_devices=NCORES, dynamic_dma_scratch_size=PIECE * 16)

    def din(name, shape, dt):
        return nc.dram_tensor(name, shape, dt, kind="ExternalInput")

    tab_in = din("tab_in", [N, D], bf16)
    xT_in = din("xT_in", [128, S], bf16)
    dinvsq_in = [din("dinvsq0_in", [128, S], bf16), din("dinvsq1_in", [128, S], bf16)]
    idx_in = [din("idx0_in", [128, max(stot0 // 16, 1)], i16),
              din("idx1_in", [128, max(stot1 // 16, 1)], i16)]
    rel_in = [din("rel0_in", [128, max(nch0, 1)], f32),
              din("rel1_in", [128, max(nch1, 1)], f32)]
    coef_in = [din("coef0_in", [128, max(nch0, 1)], f32),
               din("coef1_in", [128, max(nch1, 1)], f32)]
    wd_in = din("wd_in", [L * 2 * 128, D], bf16)
    gb_in = din("gb_in", [128, 2 * L], f32)
    wh_in = din("wh_in", [6 * 128, D], bf16)
    hb_in = din("hb_in", [128, 6], f32)
    iota_in = din("iota_in", [128, 128], bf16)
    ones_in = din("ones_in", [128, 128], bf16)

    outs = [nc.dram_tensor(n, [128, S], bf16, kind="ExternalOutput")
            for n in ("e1_o", "e2_o", "p1_o", "p2_o")]

    with tile.TileContext(nc) as tc:
        with (
            tc.tile_pool(name="const", bufs=1) as const,
            tc.tile_pool(name="g", bufs=1) as gpool,
            tc.tile_pool(name="msg", bufs=8) as msgp,
            tc.tile_pool(name="oh", bufs=64) as ohp,
            tc.tile_pool(name="scr", bufs=1) as scp,
            tc.tile_pool(name="psA", bufs=4, space="PSUM") as psA,
            tc.tile_pool(name="psB", bufs=4, space="PSUM") as psB,
            tc.tile_pool(name="dram", bufs=1, space="DRAM") as dram,
        ):
            nc.gpsimd.load_library(mlp_lib)

            # ---- persistent SBUF tiles
            iota_t = const.tile([128, 128], bf16)
            ones_t = const.tile([128, 128], bf16)
            dinvsq_t = [const.tile([128, S], bf16, tag=f"dq{t}", name=f"dq{t}")
                        for t in (0, 1)]
            idx_t = [const.tile([128, max(stot0 // 16, 1)], i16, tag="idx0", name="idx0"),
                     const.tile([128, max(stot1 // 16, 1)], i16, tag="idx1", name="idx1")]
            rel_t = [const.tile([128, max(nch0, 1)], f32, tag="rel0", name="rel0"),
                     const.tile([128, max(nch1, 1)], f32, tag="rel1", name="rel1")]
            coef_t = [const.tile([128, max(nch0, 1)], f32, tag="coef0", name="coef0"),
                      const.tile([128, max(nch1, 1)], f32, tag="coef1", name="coef1")]
            wd_t = const.tile([128, L * 2, D], bf16)     # dense weights
            wh_t = const.tile([128, 6, D], bf16)         # head weights
            whp_t = const.tile([128, 2, D], bf16)        # BN-folded emb weights
            hbf_t = const.tile([128, 2], f32)            # BN-folded emb biases
            gb_t = const.tile([128, 2 * L], f32)
            hb_t = const.tile([128, 6], f32)
            epsn_t = const.tile([128, 1], f32)
            nc.vector.memset(epsn_t[:], EPS_NORM * EPS_NORM)

            h16_t = gpool.tile([128, SP_], bf16, tag="h16")      # padded cols
            h_pre = gpool.tile([128, S], f32, tag="hpre")
            gbf_t = [gpool.tile([128, S], bf16, tag=f"gbf{t}", name=f"gbf{t}")
                     for t in (0, 1)]
            # stage (layer boundaries) and e1b (heads) never live at the same
            # time; alias them in one padded buffer to save SBUF.
            ub = gpool.tile([128, SP_], bf16, tag="stage")
            stage = ub[:].rearrange("p (w k) -> p w k", k=128)
            e1b_t = ub[:]
            t2b_t = gpool.tile([128, S], bf16, tag="t2b")
            # region-1 scatter accumulators, aliased over boundary/heads-only
            # buffers (stage/ub and t2b are idle during scatter+dense)
            grf_t = [ub[:, :S], t2b_t[:]]

            # gather-critical tensors first: layer-0 scatter starts as soon
            # as idx/rel/coef land; dinvsq/weights are only needed at dense
            nc.sync.dma_start(iota_t[:], iota_in[:])
            for t in (0, 1):
                nc.sync.dma_start(idx_t[t][:], idx_in[t][:])
                nc.sync.dma_start(rel_t[t][:], rel_in[t][:])
                nc.sync.dma_start(coef_t[t][:], coef_in[t][:])
            nc.sync.dma_start(ones_t[:], ones_in[:])
            for t in (0, 1):
                nc.sync.dma_start(dinvsq_t[t][:], dinvsq_in[t][:])
            nc.sync.dma_start(
                wd_t[:], wd_in[:].rearrange("(k p) d -> p k d", p=128))
            nc.sync.dma_start(
                wh_t[:], wh_in[:].rearrange("(k p) d -> p k d", p=128))
            nc.sync.dma_start(gb_t[:], gb_in[:])
            nc.sync.dma_start(hb_t[:], hb_in[:])
            nc.sync.dma_start(h16_t[:, :S], xT_in[:])
            if SP_ > S:
                nc.vector.memset(h16_t[:, S:], 0.0)

            # ---- internal DRAM for collectives
            ag_in = {}
            ag_out = {}
            for l in (0, 1):
                ag_in[l] = dram.tile([S, D], bf16, tag=f"agi{l}", name=f"agi{l}")
                ag_out[l] = dram.tile([N, D], bf16, addr_space="Shared",
                                      tag=f"ago{l}", name=f"ago{l}")
            st_in = [dram.tile([128, 2], f32, tag=f"sti{l}", name=f"sti{l}")
                     for l in range(L)]
            st_out = [dram.tile([128, 2], f32, addr_space="Shared",
                                tag=f"sto{l}", name=f"sto{l}") for l in range(L)]

            schs = (sch0, sch1)
            has_r1 = any(r == 1 for (r, _, _) in sch0)
            rg = [list(range(NCORES))]

            for l in range(L):
                # ---------------- scatter phase (both edge types)
                for t in (0, 1):
                    if l == 0:
                        tab_lo = tab_in[:]
                        tab_hi = tab_in[N - IDX_LIMIT:] if N > IDX_LIMIT else None
                    else:
                        tab_lo = ag_out[l - 1][:]
                        tab_hi = ag_out[l - 1][N - IDX_LIMIT:] \
                            if N > IDX_LIMIT else None

                    sch = schs[t]
                    # chunk meta: (region, window, win_first, win_last,
                    #              group_first, group_last); groups = 4 windows
                    # of one region sharing a [128,512] PSUM bank.
                    chunk_meta = []
                    for si, (r, w_, k) in enumerate(sch):
                        gf = (w_ % 4 == 0) or si == 0 or sch[si - 1][0] != r
                        gl = (w_ % 4 == 3) or si == len(sch) - 1 \
                            or sch[si + 1][0] != r
                        for j in range(k):
                            chunk_meta.append(
                                (r, w_, j == 0, j == k - 1,
                                 gf and j == 0, gl and j == k - 1))

                    pieces = _pieces(sch)
                    acc = None
                    for (p0, cnt, r) in pieces:
                        msg = msgp.tile([128, PIECE // 128, 128], bf16,
                                        tag="msg")
                        src_ap = tab_lo if r == 0 else tab_hi
                        nc.gpsimd.dma_gather(
                            msg[:, :cnt // 128, :], src_ap,
                            idx_t[t][:, p0 // 16:(p0 + cnt) // 16],
                            num_idxs=cnt, num_idxs_reg=cnt, elem_size=D,
                        )
                        for ci in range(cnt // 128):
                            gc = p0 // 128 + ci
                            (cr, w_, first, last, gfirst, glast) = chunk_meta[gc]
                            oh = ohp.tile([128, 128], bf16, tag="oh")
                            nc.vector.tensor_scalar(
                                out=oh[:], in0=iota_t[:],
                                scalar1=rel_t[t][:, gc:gc + 1],
                                scalar2=coef_t[t][:, gc:gc + 1],
                                op0=OP.is_equal, op1=OP.mult,
                            )
                            if gfirst:
                                acc = psA.tile([128, 512], f32, space="PSUM",
                                               tag="sc")
                            ws = (w_ % 4) * 128
                            nc.tensor.matmul(out=acc[:, ws:ws + 128],
                                             lhsT=msg[:, ci, :],
                                             rhs=oh[:], start=first, stop=last)
                            if glast:
                                base = (w_ // 4) * 512
                                wd = min(512, S - base)
                                # evacuate scatter PSUM on Act; each region
                                # gets its own fp16 accumulator and the dense
                                # phase sums them (extra matmul pair)
                                dst = gbf_t[t] if cr == 0 else grf_t[t]
                                nc.scalar.activation(
                                    out=dst[:, base:base + wd],
                                    in_=acc[:, :wd], func=AF.Identity)

                # ---------------- dense + stats partials
                sum_p = scp.tile([128, NF], f32, tag="sump")
                ssq_p = scp.tile([128, NF], f32, tag="ssqp")
                for ft in range(NF):
                    fw = min(512, S - ft * 512)
                    sl = slice(ft * 512, ft * 512 + fw)
                    # self-loop tiles: sl_t = dinv^2 * h  (fp16, 4x DVE)
                    slt = [scp.tile([128, 512], bf16, tag=f"sl{t}", bufs=3,
                                    name=f"sl{t}") for t in (0, 1)]
                    for t in (0, 1):
                        nc.vector.tensor_tensor(out=slt[t][:, :fw],
                                                in0=h16_t[:, sl],
                                                in1=dinvsq_t[t][:, sl],
                                                op=OP.mult)
                    dp = psB.tile([128, 512], f32, space="PSUM", tag="dense")
                    rhss = [(0, gbf_t[0][:, sl]), (1, gbf_t[1][:, sl]),
                            (0, slt[0][:, :fw]), (1, slt[1][:, :fw])]
                    if has_r1:
                        rhss += [(0, grf_t[0][:, sl]), (1, grf_t[1][:, sl])]
                    for mi, (t, rhs_ap) in enumerate(rhss):
                        nc.tensor.matmul(out=dp[:, :fw],
                                         lhsT=wd_t[:, l * 2 + t, :],
                                         rhs=rhs_ap, start=(mi == 0),
                                         stop=(mi == len(rhss) - 1))
                    nc.scalar.activation(out=h_pre[:, sl], in_=dp[:, :fw],
                                         func=AF.Identity,
                                         accum_out=sum_p[:, ft:ft + 1])
                    sq = scp.tile([128, 512], bf16, tag="sq", bufs=3)
                    nc.scalar.activation(out=sq[:, :fw], in_=dp[:, :fw],
                                         func=AF.Square,
                                         accum_out=ssq_p[:, ft:ft + 1])

                # ---------------- BN stats allreduce
                st = scp.tile([128, 2], f32, tag="st")
                nc.vector.tensor_reduce(out=st[:, 0:1], in_=sum_p[:],
                                        axis=mybir.AxisListType.X, op=OP.add)
                nc.vector.tensor_reduce(out=st[:, 1:2], in_=ssq_p[:],
                                        axis=mybir.AxisListType.X, op=OP.add)
                nc.sync.dma_start(st_in[l][:], st[:])
                nc.gpsimd.collective_compute(
                    "AllReduce", OP.add, replica_groups=rg,
                    ins=[st_in[l].opt()], outs=[st_out[l].opt()])
                if l == L - 1:
                    # raw fp16 copy of the dense output, hidden in the
                    # AllReduce window (the BN affine folds into the heads)
                    nc.scalar.activation(out=h16_t[:, :S], in_=h_pre[:],
                                         func=AF.Identity)
                sta = scp.tile([128, 2], f32, tag="sta")
                nc.sync.dma_start(sta[:], st_out[l][:])

                mean = scp.tile([128, 1], f32, tag="mean")
                var = scp.tile([128, 1], f32, tag="var")
                scl = scp.tile([128, 1], f32, tag="scl")
                sht = scp.tile([128, 1], f32, tag="sht")
                tmp = scp.tile([128, 1], f32, tag="tmp1")
                inv_n = 1.0 / float(N)
                nc.vector.tensor_scalar(out=mean[:], in0=sta[:, 0:1],
                                        scalar1=inv_n, scalar2=None, op0=OP.mult)
                nc.vector.tensor_scalar(out=var[:], in0=sta[:, 1:2],
                                        scalar1=inv_n, scalar2=None, op0=OP.mult)
                nc.vector.tensor_tensor(out=tmp[:], in0=mean[:], in1=mean[:],
                                        op=OP.mult)
                nc.vector.tensor_tensor(out=var[:], in0=var[:], in1=tmp[:],
                                        op=OP.subtract)
                # scl = gamma / sqrt(var + eps); sht = beta - mean*scl
                nc.vector.tensor_scalar(out=var[:], in0=var[:], scalar1=EPS_BN,
                                        scalar2=None, op0=OP.add)
                nc.scalar.activation(out=tmp[:], in_=var[:], func=AF.Sqrt)
                nc.vector.reciprocal(out=tmp[:], in_=tmp[:])
                nc.vector.tensor_tensor(out=scl[:], in0=gb_t[:, l:l + 1],
                                        in1=tmp[:], op=OP.mult)
                nc.vector.tensor_tensor(out=tmp[:], in0=mean[:], in1=scl[:],
                                        op=OP.mult)
                nc.vector.tensor_tensor(out=sht[:], in0=gb_t[:, L + l:L + l + 1],
                                        in1=tmp[:], op=OP.subtract)

                # ---------------- normalize (+ relu except last layer)
                if l < L - 1:
                    pass  # fused with the table build below (half-pipelined)
                else:
                    # last layer has no relu: fold the BN affine into the
                    # head weights instead of normalizing h. h16 := fp16(raw
                    # h_pre) was already copied during the AllReduce window;
                    # whp_t = wh*scl and the tanh biases absorb wh^T @ sht.
                    sht16 = scp.tile([128, 1], bf16, tag="sht16")
                    nc.vector.tensor_copy(out=sht16[:], in_=sht[:])
                    for k in (0, 1):
                        nc.scalar.activation(out=whp_t[:, k, :],
                                             in_=wh_t[:, k, :],
                                             func=AF.Identity, scale=scl[:])
                        bp = psA.tile([128, 512], f32, space="PSUM", tag="sc")
                        nc.tensor.matmul(out=bp[:, 0:1], lhsT=wh_t[:, k, :],
                                         rhs=sht16[:], start=True, stop=True)
                        nc.vector.tensor_tensor(out=hbf_t[:, k:k + 1],
                                                in0=hb_t[:, k:k + 1],
                                                in1=bp[:, 0:1], op=OP.add)

                # ---------------- next-layer table: XBAR transpose + allgather
                # normalize/XBAR/table-DMA in halves so they pipeline
                # (disjoint column ranges -> no hazards between halves)
                if l < L - 1:
                    full = (S // 128) * 128
                    halves = [(0, 3072), (3072, SP_)] if S > 3072 \
                        else [(0, SP_)]
                    for (c0, c1) in halves:
                        ce = min(c1, S)
                        nc.scalar.activation(out=h16_t[:, c0:ce],
                                             in_=h_pre[:, c0:ce],
                                             func=AF.Relu,
                                             bias=sht[:], scale=scl[:])
                        nc.sync.dma_start_transpose(
                            stage[:, c0 // 128:c1 // 128, :],
                            h16_t[:, c0:c1])
                        de = min(c1, full)
                        nc.sync.dma_start(
                            ag_in[l][c0:de].rearrange("(w p) d -> p w d",
                                                      p=128),
                            stage[:, c0 // 128:de // 128, :])
                        if c1 > full and S > full:
                            nc.sync.dma_start(
                                ag_in[l][full:],
                                stage[:S - full, S // 128, :])
                    nc.gpsimd.collective_compute(
                        "AllGather", OP.bypass, replica_groups=rg,
                        ins=[ag_in[l].opt()], outs=[ag_out[l].opt()])

            # ---------------- heads (two activation-table passes)
            # pass 1: tanh embeddings, written straight to fp16 buffers
            for ft in range(NF):
                fw = min(512, S - ft * 512)
                sl = slice(ft * 512, ft * 512 + fw)
                e1p = psB.tile([128, 512], f32, space="PSUM", tag="dense")
                nc.tensor.matmul(out=e1p[:, :fw], lhsT=whp_t[:, 0, :],
                                 rhs=h16_t[:, sl], start=True, stop=True)
                nc.scalar.activation(out=e1b_t[:, sl], in_=e1p[:, :fw],
                                     func=AF.Tanh, bias=hbf_t[:, 0:1])
                nc.sync.dma_start(outs[0][:, sl], e1b_t[:, sl])
                e2p = psB.tile([128, 512], f32, space="PSUM", tag="dense")
                nc.tensor.matmul(out=e2p[:, :fw], lhsT=whp_t[:, 1, :],
                                 rhs=h16_t[:, sl], start=True, stop=True)
                nc.scalar.activation(out=t2b_t[:, sl], in_=e2p[:, :fw],
                                     func=AF.Tanh, bias=hbf_t[:, 1:2])

            # pass 2: l2norms + projection MLPs (sqrt activation set).
            # 1/max(||x||, eps) == 1/sqrt(||x||^2 + eps^2) via the Sqrt bias.
            def inv_norm(x_ap, fw):
                # all-fp16 chain: TensorTensor only has a 2x mode and only
                # for pure 2-byte operands, so keep everything fp16
                sq16 = scp.tile([128, 512], bf16, tag="sqb", bufs=4)
                nc.vector.tensor_tensor(out=sq16[:, :fw], in0=x_ap,
                                        in1=x_ap, op=OP.mult)
                nsq = psA.tile([128, 512], f32, space="PSUM", tag="sc")
                nc.tensor.matmul(out=nsq[:, :fw], lhsT=ones_t[:],
                                 rhs=sq16[:, :fw], start=True, stop=True)
                nrm = scp.tile([128, 512], bf16, tag="nrm", bufs=4)
                nc.scalar.activation(out=nrm[:, :fw], in_=nsq[:, :fw],
                                     func=AF.Sqrt, bias=epsn_t[:])
                with nc.allow_low_precision(reason="fp16 1/norm is plenty"):
                    nc.vector.reciprocal(out=nrm[:, :fw], in_=nrm[:, :fw])
                return nrm

            # pass 2a: e2 = l2norm(tanh) scaled in place in t2b
            for ft in range(NF):
                fw = min(512, S - ft * 512)
                sl = slice(ft * 512, ft * 512 + fw)
                nrm = inv_norm(t2b_t[:, sl], fw)
                nc.vector.tensor_tensor(out=t2b_t[:, sl], in0=t2b_t[:, sl],
                                        in1=nrm[:, :fw], op=OP.mult)
                nc.sync.dma_start(outs[1][:, sl], t2b_t[:, sl])

            # pass 2b: p1 projection (2 PSUM banks/tile -> 2-tile overlap)
            for ft in range(NF):
                fw = min(512, S - ft * 512)
                sl = slice(ft * 512, ft * 512 + fw)
                r1p = psB.tile([128, 512], f32, space="PSUM", tag="dense")
                nc.tensor.matmul(out=r1p[:, :fw], lhsT=wh_t[:, 2, :],
                                 rhs=e1b_t[:, sl], start=True, stop=True)
                r1b = scp.tile([128, 512], bf16, tag="r1b", bufs=3)
                nc.scalar.activation(out=r1b[:, :fw], in_=r1p[:, :fw],
                                     func=AF.Relu, bias=hb_t[:, 2:3])
                z1p = psB.tile([128, 512], f32, space="PSUM", tag="dense")
                nc.tensor.matmul(out=z1p[:, :fw], lhsT=wh_t[:, 3, :],
                                 rhs=r1b[:, :fw], start=True, stop=True)
                z1s = scp.tile([128, 512], bf16, tag="z1s", bufs=3)
                nc.vector.tensor_scalar(out=z1s[:, :fw], in0=z1p[:, :fw],
                                        scalar1=hb_t[:, 3:4], scalar2=None,
                                        op0=OP.add)
                nrm1 = inv_norm(z1s[:, :fw], fw)
                p1s = scp.tile([128, 512], bf16, tag="p1s", bufs=3)
                nc.vector.tensor_tensor(out=p1s[:, :fw], in0=z1s[:, :fw],
                                        in1=nrm1[:, :fw], op=OP.mult)
                nc.sync.dma_start(outs[2][:, sl], p1s[:, :fw])

            # pass 2c: p2 projection from the in-place e2 in t2b
            for ft in range(NF):
                fw = min(512, S - ft * 512)
                sl = slice(ft * 512, ft * 512 + fw)
                r2p = psB.tile([128, 512], f32, space="PSUM", tag="dense")
                nc.tensor.matmul(out=r2p[:, :fw], lhsT=wh_t[:, 4, :],
                                 rhs=t2b_t[:, sl], start=True, stop=True)
                r2b = scp.tile([128, 512], bf16, tag="r2b", bufs=3)
                nc.scalar.activation(out=r2b[:, :fw], in_=r2p[:, :fw],
                                     func=AF.Relu, bias=hb_t[:, 4:5])
                z2p = psB.tile([128, 512], f32, space="PSUM", tag="dense")
                nc.tensor.matmul(out=z2p[:, :fw], lhsT=wh_t[:, 5, :],
                                 rhs=r2b[:, :fw], start=True, stop=True)
                z2s = scp.tile([128, 512], bf16, tag="z2s", bufs=3)
                nc.vector.tensor_scalar(out=z2s[:, :fw], in0=z2p[:, :fw],
                                        scalar1=hb_t[:, 5:6], scalar2=None,
                                        op0=OP.add)
                nrm2 = inv_norm(z2s[:, :fw], fw)
                p2s = scp.tile([128, 512], bf16, tag="p2s", bufs=3)
                nc.vector.tensor_tensor(out=p2s[:, :fw], in0=z2s[:, :fw],
                                        in1=nrm2[:, :fw], op=OP.mult)
                nc.sync.dma_start(outs[3][:, sl], p2s[:, :fw])

    nc.compile()
    return nc


# ---------------------------------------------------------------- entry point

def _run(inputs, trace=False, trace_kwargs=None, nc_out=None):
    x = np.asarray(inputs["x"], np.float32)
    N = x.shape[0]
    assert N % NCORES == 0
    S = N // NCORES

    d0 = _prep_type(inputs["edge_index0"], N, S)
    d1 = _prep_type(inputs["edge_index1"], N, S)
    (dinv0, dinvsq0, sch0, idx0, rel0, coef0, stot0, nch0) = d0
    (dinv1, dinvsq1, sch1, idx1, rel1, coef1, stot1, nch1) = d1

    nc = _build(N, S, sch0, stot0, nch0, sch1, stot1, nch1)
    if nc_out is not None:
        nc_out.append(nc)

    tab = x.astype(np.float16)

    W0 = np.asarray(inputs["W0"], np.float32)
    W1 = np.asarray(inputs["W1"], np.float32)
    wd = np.zeros((L * 2 * 128, D), np.float32)
    for l in range(L):
        wd[(l * 2) * 128:(l * 2 + 1) * 128] = W0[l]
        wd[(l * 2 + 1) * 128:(l * 2 + 2) * 128] = W1[l]
    gb = np.stack([np.asarray(inputs["gamma"], np.float32).T,
                   np.asarray(inputs["beta"], np.float32).T], 0)
    gb = np.concatenate([gb[0], gb[1]], axis=1)  # [128, 2L]
    wh = np.concatenate([np.asarray(inputs[k], np.float32) for k in
                         ("emb1_W", "emb2_W", "ph1_Wa", "ph1_Wb",
                          "ph2_Wa", "ph2_Wb")], 0)
    hb = np.stack([np.asarray(inputs[k], np.float32) for k in
                   ("emb1_b", "emb2_b", "ph1_ba", "ph1_bb",
                    "ph2_ba", "ph2_bb")], 1)

    iota = np.broadcast_to(np.arange(128, dtype=np.float32),
                           (128, 128)).astype(np.float16)
    ones = np.ones((128, 128), np.float16)

    in_maps = []
    for c in range(NCORES):
        sl = slice(c * S, (c + 1) * S)
        in_maps.append({
            "tab_in": tab,
            "xT_in": np.ascontiguousarray(x[sl].T).astype(np.float16),
            "dinvsq0_in": np.ascontiguousarray(
                np.broadcast_to(dinvsq0[sl], (128, S))).astype(np.float16),
            "dinvsq1_in": np.ascontiguousarray(
                np.broadcast_to(dinvsq1[sl], (128, S))).astype(np.float16),
            "idx0_in": idx0[c], "idx1_in": idx1[c],
            "rel0_in": rel0[c], "rel1_in": rel1[c],
            "coef0_in": coef0[c], "coef1_in": coef1[c],
            "wd_in": wd.astype(np.float16),
            "gb_in": gb, "wh_in": wh.astype(np.float16), "hb_in": hb,
            "iota_in": iota, "ones_in": ones,
        })

    res = run_bass_kernel_spmd(nc, in_maps, list(range(NCORES)),
                               trace=trace, **(trace_kwargs or {}))

    full = {}
    for name in ("e1_o", "e2_o", "p1_o", "p2_o"):
        full[name] = np.concatenate(
            [res.results[c][name].T.astype(np.float32)
             for c in range(NCORES)], axis=0)
    return (full["e1_o"], full["e2_o"], full["p1_o"], full["p2_o"]), res


def kernel(**inputs):
    out, _ = _run(inputs)
    return out
